# revision 31
# baseline (speedup 1.0000x reference)
"""Trainium2 Bass kernel for the AttentionBlock problem.

Fixed problem shape: x [4, 64, 64, 64] fp32, GroupNorm(32 groups) ->
1x1 conv Q/K/V -> softmax(Q^T K / 8) -> V @ attn^T -> 1x1 conv + residual.

Sharding: 8 cores, core = 2*batch + query_half. Each core holds its batch's
full x (for K/V) and computes outputs for its 2048-query half.

Layout strategy (per core):
  - x, K, Q, V live as [c=64 partitions, n free]; matmul operands in bf16
    with fp32 PSUM accumulation (attention is ~5% of the residual output,
    so final norm-rel-err stays ~1e-3).
  - GroupNorm stats via bn_stats + a DVE 32x32 stream-transpose pair-combine
    (PE-free); the affine fold goes INTO the projection weights
    (W*diag(s) stationaries) and a bias row (t/s)^T@(W*s)+b against a
    host-appended ones-row of x, so there is no normalization pass at all.
  - Scores are computed TRANSPOSED: S_T[k,q] = K_blk^T Q (contract c on
    partitions); the softmax denominator comes free from a ones-column
    appended to V^T during the PV matmul (no cross-partition reductions).
  - exp() runs on ScalarE directly out of PSUM in 1024-wide ops, no max
    subtraction (scores are O(+-10) here; exp stays well inside fp32 range).
  - PE stationary operands switch only twice per key block (K_blk for 4
    score matmuls, V^T_blk for 4 PV matmuls); PV for block kb-1 is emitted
    after the scores of kb (software pipeline).
  - V^T blocks and the output-projection transposes go through the DMA xbar
    transpose (bf16, 128B-aligned dst offsets), keeping them off the PE.
  - The PE clock on this part ramps 1.2->2.4 GHz only after sustained
    uninterrupted activity and re-throttles after idle gaps; a dummy
    same-weight warmup burst keeps the PE busy from kernel start through
    the projections so the attention loop can run warm (~125 us) instead
    of cold (~177 us) when the chip's power state cooperates.
  - Tail: augmented 65x65 Wo carries the denominators through the output
    projection; one xbar transpose puts q on partitions, one batched
    reciprocal + 16 fused multiply-adds apply 1/denom + residual + bias,
    one strided DMA writes y back.
"""

import numpy as np
import ml_dtypes

import concourse.bass as bass
import concourse.mybir as mybir
import concourse.tile as tile
from concourse.tile_rust import add_dep_helper
from concourse.vector_clock import ScopedClock

B, C, H, W = 4, 64, 64, 64
N = H * W            # 4096
NQ = N // 2          # queries per core
EPS = 1e-5
KB = 32              # key blocks of 128
WARMUP_REPS = 48     # initial PE warmup burst (more interleaved later)
F32 = mybir.dt.float32
BF16 = mybir.dt.bfloat16
AF = mybir.ActivationFunctionType
ALU = mybir.AluOpType


# ---------------------------------------------------------------------------
# This container's walrus codegen rejects >1 sync wait on one instruction
# ("Too many sync wait commands") — split extra waits onto preceding same-
# engine NOPs (engines execute in order, so semantics are preserved), and do
# the same for the TileContext tail drain.
def _install_drain_patch():
    if getattr(tile.TileContext, "_drain_patch_installed", False):
        return

    orig_commit = tile.TileContext._commit_instruction

    def _split_commit(self, inst, lazy_reg_writes=True):
        si = getattr(inst, "sync_info", None)
        if (
            si is not None
            and len(si.on_wait) > 1
            and inst.engine != mybir.EngineType.Unassigned
        ):
            waits = list(si.on_wait)
            inst.sync_info = mybir.SyncInfo(
                on_wait=waits[-1:], on_update=list(si.on_update)
            )
            for w in waits[:-1]:
                nop = mybir.InstNoOp(
                    name=self.nc.get_next_instruction_name(),
                    sync_info=mybir.SyncInfo(on_wait=[w], on_update=[]),
                    bass_nofuse=True,
                    engine=inst.engine,
                )
                orig_commit(self, nop, lazy_reg_writes=False)
        orig_commit(self, inst, lazy_reg_writes)

    def _patched(self, tick_clock, wait_clock):
        nc = self.nc
        drain_inst = nc.sync.drain()
        wait_clock.add_sem_waits(
            drain_inst.ins, ScopedClock({None: tick_clock.global_clock})
        )
        si = drain_inst.ins.sync_info
        if si is not None and len(si.on_wait) > 1:
            waits = list(si.on_wait)
            drain_inst.ins.sync_info = mybir.SyncInfo(
                on_wait=waits[:1], on_update=list(si.on_update)
            )
            for i in range(1, len(waits)):
                extra = nc.sync.drain()
                extra.ins.sync_info = mybir.SyncInfo(
                    on_wait=waits[i : i + 1], on_update=[]
                )
        nc.all_engine_barrier()
        assert self.sems is not None
        popped = nc._tile_sem_poison_stack.pop()
        assert popped is self._sem_poison
        nc.clear_and_free_semaphores(list(self.sems.allocated().values()))
        nc.all_engine_barrier()

    tile.TileContext._commit_instruction = _split_commit
    tile.TileContext._drain_and_barrier = _patched
    tile.TileContext._drain_patch_installed = True


def build_nc():
    _install_drain_patch()
    nc = bass.Bass()

    # per-core data
    # x / xq carry a host-appended ones row (row 64) for the bias-row trick
    x_d = nc.dram_tensor("x", [C + 1, N], F32, kind="ExternalInput")
    xq_d = nc.dram_tensor("xq", [C + 1, NQ], F32, kind="ExternalInput")
    xt_d = nc.dram_tensor("xt", [NQ, C], F32, kind="ExternalInput")
    # one zero row; DMA-broadcast (0-stride partition reads) zero-fills the
    # padded partition ranges without touching the DVE critical path
    zp_d = nc.dram_tensor("zpad", [1, N], BF16, kind="ExternalInput")
    # replicated weights / constants
    wqk_d = nc.dram_tensor("w_qk", [C, 2 * C], BF16, kind="ExternalInput")
    wv_d = nc.dram_tensor("wv_t", [C, C], BF16, kind="ExternalInput")
    waug_d = nc.dram_tensor("w_aug", [C + 1, C + 1], BF16, kind="ExternalInput")
    # gbias columns: 0 gamma, 1 beta, 2 [bq;bk] stacked, 3 bv
    gb_d = nc.dram_tensor("gbias", [2 * C, 4], F32, kind="ExternalInput")
    brow_d = nc.dram_tensor("brows", [1, 3 * C], F32, kind="ExternalInput")
    bo_d = nc.dram_tensor("bo_bc", [128, C], F32, kind="ExternalInput")
    y_d = nc.dram_tensor("y", [NQ, C], F32, kind="ExternalOutput")

    with tile.TileContext(nc) as tc:
        with (
            tc.tile_pool(name="const", bufs=1) as const,
            tc.tile_pool(name="big", bufs=1) as big,
            tc.tile_pool(name="stats", bufs=2) as stats,
            tc.tile_pool(name="pt", bufs=4) as ptp,
            tc.tile_pool(name="tail", bufs=2) as tailp,
            tc.tile_pool(name="yp", bufs=3) as yp,
            tc.tile_pool(name="xtp", bufs=3) as xtp,
            tc.tile_pool(name="sps", bufs=2, space="PSUM") as sps,
            tc.tile_pool(name="ops", bufs=4, space="PSUM") as ops,
        ):
            # ---- load constants
            wqk = const.tile([C, 2 * C], BF16, tag="wqk")
            wv = const.tile([C, C], BF16, tag="wv")
            waug = const.tile([128, 128], BF16, tag="waug")
            gb = const.tile([2 * C, 4], F32, tag="gb")
            brow = const.tile([1, 3 * C], F32, tag="brow")
            bo_bc = const.tile([128, C], F32, tag="bo")
            gamma = gb[:C, 0:1]
            beta = gb[:C, 1:2]
            bqk_col = gb[:, 2:3]
            bv_col = gb[:C, 3:4]

            # ---- PE clock warmup: the PE clock ramps 1.2->2.4 GHz only
            # after sustained uninterrupted activity and then stays warm as
            # long as it never idles >~3us. Keep the PE busy with dummy
            # same-weight matmuls from kernel start until the projections are
            # ready (more reps are interleaved into the projection phase).
            warm_sb = const.tile([128, 512], BF16, tag="warm")
            nc.vector.memset(warm_sb, 0.0)

            def warm_reps(n, base):
                for i in range(n):
                    wp = ops.tile([128, 512], F32, tag="o", name=f"w{base}_{i}")
                    nc.tensor.matmul(
                        out=wp, lhsT=warm_sb[:, 0:128], rhs=warm_sb,
                        start=True, stop=True,
                    )

            warm_reps(WARMUP_REPS, "a")

            # chain-independent prep, off the GroupNorm critical path
            eps_t = stats.tile([C, 1], F32, tag="eps")
            nc.vector.memset(eps_t, EPS)
            # first ACT instruction = tiny Exp: walrus attaches the one-time
            # ~2.7us ACT_TABLE_LOAD here, in the startup dead zone, instead
            # of stalling the first real activation mid-kernel
            tblw = stats.tile([C, 1], F32, tag="tblw")
            nc.scalar.activation(out=tblw, in_=eps_t, func=AF.Exp)
            tp1 = stats.tile([C, 32], F32, tag="tp1")
            nc.vector.memset(tp1, 0.0)
            g_inv = stats.tile([C, 1], F32, tag="ginv")
            nc.vector.memset(waug, 0.0)

            # ---- load x; bn_stats per 512-chunk as chunks arrive.
            # bf16 copies of x/xq for the projection matmuls come via
            # gpsimd cast-DMAs (GroupNorm scale/shift are folded into the
            # projection weights, so no separate normalization pass).
            x_sb = big.tile([C + 1, N], F32, tag="x")
            xq_sb = big.tile([C + 1, NQ], F32, tag="xq")
            # projection inputs padded to 128 partitions (rows 65:128 zero)
            # so every matmul enables the full 128x128 array: the HAM clock
            # gate integrates the ENABLED array fraction, and half-array
            # matmuls (contract=64 / 65 out rows) get throttled to 1.2 GHz
            # even when the PE has zero idle gaps.
            x_bf = big.tile([128, N], BF16, tag="xbf")
            xq_bf = big.tile([128, NQ], BF16, tag="xqbf")

            def zpad(parts, *shape):
                src = zp_d[0:1, 0 : shape[-1]].partition_broadcast(parts)
                if len(shape) == 1:
                    return src.squeeze()
                return src.broadcast_to((parts, *shape))
            st = stats.tile([C, 8, 6], F32, tag="bnst")
            for j in range(8):
                sl = bass.ts(j, 512)
                eng = nc.sync if j % 2 else nc.scalar
                eng.dma_start(out=x_sb[:, sl], in_=x_d[:, sl])
                nc.vector.bn_stats(out=st[:, j, :], in_=x_sb[0:C, sl])
                # bf16 cast for the projection inputs on ScalarE (idle
                # here; gpsimd casts are ~3x slower and stall concurrent
                # DVE ops via port sharing)
                nc.scalar.copy(out=x_bf[0 : C + 1, sl], in_=x_sb[:, sl])
            for j in range(4):
                sl = bass.ts(j, 512)
                eng = nc.scalar if j % 2 else nc.sync
                eng.dma_start(out=xq_sb[:, sl], in_=xq_d[:, sl])
                nc.scalar.copy(out=xq_bf[0 : C + 1, sl], in_=xq_sb[:, sl])
            nc.sync.dma_start(out=x_bf[C:128, :], in_=zpad(64, N))
            nc.sync.dma_start(out=xq_bf[C:128, :], in_=zpad(64, NQ))
            nc.gpsimd.dma_start(out=wqk, in_=wqk_d[:, :])
            nc.gpsimd.dma_start(out=wv, in_=wv_d[:, :])
            nc.gpsimd.dma_start(out=waug[0 : C + 1, 0 : C + 1], in_=waug_d[:, :])
            nc.gpsimd.dma_start(out=gb, in_=gb_d[:, :])
            nc.gpsimd.dma_start(out=brow, in_=brow_d[:, :])
            nc.gpsimd.dma_start(out=bo_bc, in_=bo_d[:, :])
            mv = stats.tile([C, 2], F32, tag="mv")
            nc.vector.bn_aggr(out=mv, in_=st)
            # me2 = [mean, var + mean^2] per channel
            me2 = stats.tile([C, 2], F32, tag="me2")
            nc.vector.tensor_copy(out=me2[:, 0:1], in_=mv[:, 0:1])
            m2 = stats.tile([C, 1], F32, tag="m2")
            nc.vector.tensor_mul(out=m2, in0=mv[:, 0:1], in1=mv[:, 0:1])
            nc.vector.tensor_add(out=me2[:, 1:2], in0=mv[:, 1:2], in1=m2)
            # group (channel-pair) sums of [mean, E[x^2]] without touching
            # the PE: 32x32 stream-transpose, add adjacent columns, replicate,
            # transpose back.
            nc.vector.tensor_copy(out=tp1[:, 0:2], in_=me2)
            tp2 = stats.tile([C, 32], F32, tag="tp2")
            nc.vector.transpose(out=tp2, in_=tp1)
            t2v = tp2.rearrange("p (g two) -> p g two", two=2)
            tp3 = stats.tile([C, 16], F32, tag="tp3")
            nc.vector.tensor_add(out=tp3, in0=t2v[:, :, 0], in1=t2v[:, :, 1])
            tp4 = stats.tile([C, 32], F32, tag="tp4")
            t4v = tp4.rearrange("p (g two) -> p g two", two=2)
            nc.vector.tensor_copy(out=t4v[:, :, 0], in_=tp3)
            nc.vector.tensor_copy(out=t4v[:, :, 1], in_=tp3)
            tp5 = stats.tile([C, 32], F32, tag="tp5")
            nc.vector.transpose(out=tp5, in_=tp4)
            # tp5[:, 0] = 2*mean_g, tp5[:, 1] = 2*E[x^2]_g per channel
            mean_g = stats.tile([C, 1], F32, tag="meang")
            nc.vector.tensor_scalar(
                out=mean_g, in0=tp5[:, 0:1], scalar1=0.5, scalar2=None,
                op0=ALU.mult,
            )
            varg = stats.tile([C, 1], F32, tag="varg")
            nc.vector.tensor_mul(out=varg, in0=mean_g, in1=mean_g)
            nc.vector.scalar_tensor_tensor(
                out=varg, in0=tp5[:, 1:2], scalar=0.5, in1=varg,
                op0=ALU.mult, op1=ALU.subtract,
            )
            # rstd = 1/sqrt(var+eps);  s = rstd*gamma;  t = beta - mean*s
            nc.scalar.activation(out=varg, in_=varg, func=AF.Sqrt, bias=eps_t)

            rstd = stats.tile([C, 1], F32, tag="rstd")
            nc.vector.reciprocal(out=rstd, in_=varg)
            s_col = stats.tile([C, 1], F32, tag="scol")
            nc.vector.tensor_mul(out=s_col, in0=rstd, in1=gamma)
            t_col = stats.tile([C, 1], F32, tag="tcol")
            nc.vector.tensor_mul(out=t_col, in0=mean_g, in1=s_col)
            nc.vector.tensor_tensor(out=t_col, in0=beta, in1=t_col, op=ALU.subtract)

            # fold the GroupNorm affine into the projections:
            #   W @ (x*s + t) = (W*diag(s)) @ x + (W @ t)
            # the W@t bias goes in as a 65th contraction row against the
            # ones-row in x_bf/xq_bf, so projection copies are bias-free
            wqk_s = const.tile([128, 2 * C], BF16, tag="wqks")
            nc.vector.memset(wqk_s[C:128, :], 0.0)
            nc.vector.tensor_scalar_mul(out=wqk_s[0:C, :], in0=wqk, scalar1=s_col)
            wv_s = const.tile([128, 128], BF16, tag="wvs")
            nc.vector.memset(wv_s, 0.0)
            nc.vector.tensor_scalar_mul(
                out=wv_s[0:C, 0:C], in0=wv, scalar1=s_col
            )
            # bias matmuls reuse the SCALED stationaries: W@t = (W*s)@(t/s)
            # t/s = t * std / gamma   (varg holds sqrt(var+eps) here)
            nc.vector.reciprocal(out=g_inv, in_=gamma)
            s_inv = stats.tile([C, 1], F32, tag="sinv")
            nc.vector.tensor_mul(out=s_inv, in0=varg, in1=g_inv)
            t_bf = stats.tile([C, 1], BF16, tag="tbf")
            nc.vector.tensor_mul(out=t_bf, in0=t_col, in1=s_inv)

            # ---- QK fused pass over xn (rows 0:64 = Q+bq, 64:128 = K+bk),
            #      V pass, all with one stationary each
            # K and Q padded to 128 partitions (zeros below row 64) so the
            # score matmuls contract over the full 128 array rows
            k_sb = big.tile([128, N], BF16, tag="k")
            q_sb = big.tile([128, NQ], BF16, tag="q")
            v_sb = big.tile([C, N], BF16, tag="v")
            nc.sync.dma_start(out=k_sb[C:128, :], in_=zpad(64, N))
            nc.sync.dma_start(out=q_sb[C:128, :], in_=zpad(64, NQ))
            # bias rows: (t/s)^T @ (W*s) + b  -> row 64 of each stationary
            trow_ps = sps.tile([1, 2 * C], F32, tag="sps", name="trowqk")
            nc.tensor.matmul(
                out=trow_ps, lhsT=t_bf, rhs=wqk_s[0:C, :], start=True, stop=True
            )
            nc.vector.tensor_add(
                out=wqk_s[C : C + 1, :], in0=trow_ps, in1=brow[0:1, 0 : 2 * C]
            )
            tvow_ps = sps.tile([1, C], F32, tag="sps", name="trowv")
            nc.tensor.matmul(
                out=tvow_ps, lhsT=t_bf, rhs=wv_s[0:C, 0:C], start=True, stop=True
            )
            nc.vector.tensor_add(
                out=wv_s[C : C + 1, 0:C], in0=tvow_ps,
                in1=brow[0:1, 2 * C : 3 * C],
            )
            # one wqk_s-stationary run: 8 chunks of x (K rows) + 4 of xq (Q),
            # pure copies strictly alternating ACT/DVE so the PE never stalls
            for j in range(8):
                sl = bass.ts(j, 512)
                ps = ops.tile([128, 512], F32, tag="o", name=f"qk{j}")
                nc.tensor.matmul(
                    out=ps, lhsT=wqk_s, rhs=x_bf[:, sl], start=True, stop=True
                )
                if j % 2:
                    nc.scalar.activation(
                        out=k_sb[0:C, sl], in_=ps[64:128, :], func=AF.Copy
                    )
                else:
                    nc.vector.tensor_copy(out=k_sb[0:C, sl], in_=ps[64:128, :])
            for j in range(4):
                sl = bass.ts(j, 512)
                ps = ops.tile([128, 512], F32, tag="o", name=f"qq{j}")
                nc.tensor.matmul(
                    out=ps, lhsT=wqk_s, rhs=xq_bf[:, sl], start=True, stop=True
                )
                if j % 2:
                    nc.scalar.activation(
                        out=q_sb[0:C, sl], in_=ps[0:64, :], func=AF.Copy
                    )
                else:
                    nc.vector.tensor_copy(out=q_sb[0:C, sl], in_=ps[0:64, :])
            # then one wv_s-stationary run; V^T xbar transposes per pair so
            # the first key blocks are ready as the attention loop starts
            for j in range(8):
                sl = bass.ts(j, 512)
                ps = ops.tile([128, 512], F32, tag="o", name=f"v{j}")
                nc.tensor.matmul(
                    out=ps, lhsT=wv_s, rhs=x_bf[:, sl], start=True, stop=True
                )
                if j % 2:
                    nc.scalar.activation(
                        out=v_sb[:, sl], in_=ps[0:C, :], func=AF.Copy
                    )
                else:
                    nc.vector.tensor_copy(out=v_sb[:, sl], in_=ps[0:C, :])

            # ---- V^T blocks [128, 65] with ones column, via DMA xbar
            # per-block stride padded to 128 elements: the xbar transpose
            # needs 128B-aligned destination offsets. out[p, kb, c] =
            # V^T[kb*128 + p, c]; 4 chunked calls so early key blocks are
            # ready as soon as their v chunks are copied.
            vt = big.tile([128, KB, 128], BF16, tag="vt")
            # zero-pad stationary cols C+1:128 so PV matmuls enable all four
            # 32-col groups of the array (out rows 65:128 accumulate zeros)
            nc.sync.dma_start(
                out=vt[:, :, C + 1 : 128], in_=zpad(128, KB, 63)
            )
            for t in range(4):
                nc.sync.dma_start_transpose(
                    out=vt[:, bass.ds(t * 8, 8), 0:C],
                    in_=v_sb[:, bass.ts(t, 1024)],
                )
            nc.vector.memset(vt[:, :, C : C + 1], 1.0)

            # ---- whole-xt load up front (tail residual input)
            xt_all = xtp.tile([128, 16, C], F32, tag="xt", bufs=1)
            nc.sync.dma_start(
                out=xt_all, in_=xt_d.rearrange("(j p) c -> p j c", p=128)
            )

            # ---- main attention loop
            o_tiles = [
                ops.tile([128, 512], F32, tag="o", name=f"o{qc}")
                for qc in range(4)
            ]
            # software-pipelined: PV for block kb-1 is emitted after the score
            # matmuls for block kb, so the PE does 4 same-stationary matmuls
            # per weight switch and exp(kb-1) has a full block to finish.
            def emit_pv(kb, p2, after):
                for qc in range(4):
                    mm = nc.tensor.matmul(
                        out=o_tiles[qc], lhsT=vt[:, kb, 0:128],
                        rhs=p2[qc // 2][:, (qc % 2) * 512 : (qc % 2 + 1) * 512],
                        start=(kb == 0), stop=(kb == KB - 1),
                        skip_group_check=True,
                    )
                    if qc == 0 and after is not None:
                        # keep the PE stream in same-stationary runs of 4:
                        # PV(kb-1) only after the last score matmul of kb
                        add_dep_helper(
                            mm.ins, after.ins, sync=False,
                            reason="group PE same-stationary runs",
                        )

            prev = None
            for kb in range(KB):
                kblk = k_sb[:, bass.ts(kb, 128)]
                s2 = []
                last_s = None
                for h in range(2):
                    sp = sps.tile([128, 1024], F32, tag="sps", name=f"s{kb}_{h}")
                    nc.tensor.matmul(
                        out=sp[:, 0:512], lhsT=kblk,
                        rhs=q_sb[:, bass.ds(h * 1024, 512)],
                        start=True, stop=True,
                    )
                    last_s = nc.tensor.matmul(
                        out=sp[:, 512:1024], lhsT=kblk,
                        rhs=q_sb[:, bass.ds(h * 1024 + 512, 512)],
                        start=True, stop=True,
                    )
                    s2.append(sp)
                p2 = []
                for h in range(2):
                    p = ptp.tile([128, 1024], BF16, tag="p", name=f"p{kb}_{h}")
                    nc.scalar.activation(out=p, in_=s2[h], func=AF.Exp, scale=0.125)
                    p2.append(p)
                if kb in (1, 2):
                    # pad the PE over exp(0)'s table load + latency so the
                    # pipeline fill doesn't leave a clock-dropping idle gap
                    warm_reps(3, f"fill{kb}")
                if prev is not None:
                    emit_pv(kb - 1, prev, last_s)
                prev = p2
            emit_pv(KB - 1, prev, None)

            # ---- tail: project through augmented Wo, DMA-transpose,
            #      normalize by denominator, add residual + bo, store
            # xt was loaded up front (xt_all); add bo once, broadcast over j
            xtb = xtp.tile([128, 16, C], F32, tag="xtb", bufs=1)
            bo_bcast = bass.AP(
                tensor=bo_bc.tensor, offset=bo_bc.offset,
                ap=[list(bo_bc.ap[0]), [0, 16], list(bo_bc.ap[1])],
            )
            nc.vector.tensor_add(out=xtb, in0=xt_all, in1=bo_bcast)

            # tail in two halves so transpose/normalize/store of half 0
            # overlap the z-projection of half 1
            z_all = tailp.tile([80, 2048], BF16, tag="z", bufs=1)
            zt_all = tailp.tile([128, 16, 128], BF16, tag="zt", bufs=1)
            r_all = yp.tile([128, 16], F32, tag="r", bufs=1)
            y_all = yp.tile([128, 16, C], F32, tag="y", bufs=1)
            y_view = y_d.rearrange("(j p) c -> p j c", p=128)
            for hh in range(2):
                for qc in (2 * hh, 2 * hh + 1):
                    ou = tailp.tile([128, 512], BF16, tag="ou")
                    if qc % 2:
                        nc.vector.tensor_copy(out=ou, in_=o_tiles[qc])
                    else:
                        nc.scalar.activation(
                            out=ou, in_=o_tiles[qc], func=AF.Copy
                        )
                    z_ps = sps.tile([128, 512], F32, tag="sps", name=f"z{qc}")
                    nc.tensor.matmul(
                        out=z_ps, lhsT=waug, rhs=ou, start=True, stop=True
                    )
                    if qc % 2:
                        nc.scalar.activation(
                            out=z_all[0 : C + 1, bass.ts(qc, 512)],
                            in_=z_ps[0 : C + 1, :],
                            func=AF.Copy,
                        )
                    else:
                        nc.vector.tensor_copy(
                            out=z_all[0 : C + 1, bass.ts(qc, 512)],
                            in_=z_ps[0 : C + 1, :],
                        )
                heng = nc.scalar if hh else nc.sync
                jsl = bass.ds(hh * 8, 8)
                heng.dma_start_transpose(
                    out=zt_all[:, jsl, 0:80],
                    in_=z_all[:, bass.ds(hh * 1024, 1024)],
                )
                nc.vector.reciprocal(
                    out=r_all[:, jsl], in_=zt_all[:, jsl, C]
                )
                for j in range(8 * hh, 8 * hh + 8):
                    nc.vector.scalar_tensor_tensor(
                        out=y_all[:, j, :], in0=zt_all[:, j, 0:C],
                        scalar=r_all[:, j : j + 1], in1=xtb[:, j, :],
                        op0=ALU.mult, op1=ALU.add,
                    )
                heng.dma_start(out=y_view[:, jsl, :], in_=y_all[:, jsl, :])
    return nc


_NC = None


def _get_nc():
    global _NC
    if _NC is None:
        _NC = build_nc()
    return _NC


def _prep_maps(x, Wq, bq, Wk, bk, Wv, bv, Wo, bo, gamma, beta):
    bf = ml_dtypes.bfloat16
    w_qk = np.concatenate([Wq.T, Wk.T], axis=1).astype(bf)  # [cin, 2c]
    wv_t = np.ascontiguousarray(Wv.T).astype(bf)
    w_aug = np.zeros((C + 1, C + 1), np.float32)
    w_aug[:C, :C] = Wo.T
    w_aug[C, C] = 1.0
    w_aug = w_aug.astype(bf)
    gbias = np.zeros((2 * C, 4), np.float32)
    gbias[:C, 0] = gamma
    gbias[:C, 1] = beta
    gbias[:, 2] = np.concatenate([bq, bk])
    gbias[:C, 3] = bv
    brows = np.zeros((1, 3 * C), np.float32)
    brows[0, : 2 * C] = np.concatenate([bq, bk])
    brows[0, 2 * C :] = bv
    bo_bc = np.tile(bo[None, :], (128, 1)).astype(np.float32)

    shared = dict(
        w_qk=w_qk, wv_t=wv_t, w_aug=w_aug, gbias=gbias, brows=brows,
        bo_bc=bo_bc, zpad=np.zeros((1, N), bf),
    )
    in_maps = []
    for core in range(8):
        b, half = core // 2, core % 2
        xm = np.ascontiguousarray(x[b].reshape(C, N)).astype(np.float32)
        ones = np.ones((1, N), np.float32)
        xm1 = np.concatenate([xm, ones], axis=0)
        xqm = np.ascontiguousarray(xm1[:, half * NQ : (half + 1) * NQ])
        xtm = np.ascontiguousarray(xm.T[half * NQ : (half + 1) * NQ, :])
        in_maps.append(dict(shared, x=xm1, xq=xqm, xt=xtm))
    return in_maps


def run(inputs, trace=False):
    from concourse.bass_utils import run_bass_kernel_spmd

    inputs = {k: np.asarray(v) for k, v in inputs.items()}
    nc = _get_nc()
    in_maps = _prep_maps(**inputs)
    res = run_bass_kernel_spmd(
        nc, in_maps, core_ids=list(range(8)), trace=trace
    )
    out = np.empty((B, C, N), np.float32)
    for core in range(8):
        b, half = core // 2, core % 2
        out[b][:, half * NQ : (half + 1) * NQ] = res.results[core]["y"].T
    return out.reshape(B, C, H, W), res


def kernel(**inputs):
    out, _ = run(inputs, trace=False)
    return out



# revision 35
# speedup vs baseline: 1.4174x; 1.4174x over previous
"""Trainium2 Bass kernel for the AttentionBlock problem.

Fixed problem shape: x [4, 64, 64, 64] fp32, GroupNorm(32 groups) ->
1x1 conv Q/K/V -> softmax(Q^T K / 8) -> V @ attn^T -> 1x1 conv + residual.

Sharding: 8 cores, core = 2*batch + query_half. Each core holds its batch's
full x (for K/V) and computes outputs for its 2048-query half.

Layout strategy (per core):
  - x, K, Q, V live as [c=64 partitions, n free]; matmul operands in bf16
    with fp32 PSUM accumulation (attention is ~5% of the residual output,
    so final norm-rel-err stays ~1e-3).
  - GroupNorm stats via bn_stats + a DVE 32x32 stream-transpose pair-combine
    (PE-free); the affine fold goes INTO the projection weights
    (W*diag(s) stationaries) and a bias row (t/s)^T@(W*s)+b against a
    host-appended ones-row of x, so there is no normalization pass at all.
  - Scores are computed TRANSPOSED: S_T[k,q] = K_blk^T Q (contract c on
    partitions); the softmax denominator comes free from a ones-column
    appended to V^T during the PV matmul (no cross-partition reductions).
  - exp() runs on ScalarE directly out of PSUM in 1024-wide ops, no max
    subtraction (scores are O(+-10) here; exp stays well inside fp32 range).
  - PE stationary operands switch only twice per key block (K_blk for 4
    score matmuls, V^T_blk for 4 PV matmuls); PV for block kb-1 is emitted
    after the scores of kb (software pipeline).
  - V^T blocks and the output-projection transposes go through the DMA xbar
    transpose (bf16, 128B-aligned dst offsets), keeping them off the PE.
  - The PE clock on this part ramps 1.2->2.4 GHz only after sustained
    uninterrupted activity and re-throttles after idle gaps; a dummy
    same-weight warmup burst keeps the PE busy from kernel start through
    the projections so the attention loop can run warm (~125 us) instead
    of cold (~177 us) when the chip's power state cooperates.
  - Tail: augmented 65x65 Wo carries the denominators through the output
    projection; one xbar transpose puts q on partitions, one batched
    reciprocal + 16 fused multiply-adds apply 1/denom + residual + bias,
    one strided DMA writes y back.
"""

import numpy as np
import ml_dtypes

import concourse.bass as bass
import concourse.mybir as mybir
import concourse.tile as tile
from concourse.tile_rust import add_dep_helper
from concourse.vector_clock import ScopedClock

B, C, H, W = 4, 64, 64, 64
N = H * W            # 4096
NQ = N // 2          # queries per core
EPS = 1e-5
KB = 32              # key blocks of 128
WARMUP_REPS = 30     # initial PE warmup burst (more interleaved later)
F32 = mybir.dt.float32
BF16 = mybir.dt.bfloat16
AF = mybir.ActivationFunctionType
ALU = mybir.AluOpType


# ---------------------------------------------------------------------------
# This container's walrus codegen rejects >1 sync wait on one instruction
# ("Too many sync wait commands") — split extra waits onto preceding same-
# engine NOPs (engines execute in order, so semantics are preserved), and do
# the same for the TileContext tail drain.
def _install_drain_patch():
    if getattr(tile.TileContext, "_drain_patch_installed", False):
        return

    orig_commit = tile.TileContext._commit_instruction

    def _split_commit(self, inst, lazy_reg_writes=True):
        si = getattr(inst, "sync_info", None)
        if (
            si is not None
            and len(si.on_wait) > 1
            and inst.engine != mybir.EngineType.Unassigned
        ):
            waits = list(si.on_wait)
            inst.sync_info = mybir.SyncInfo(
                on_wait=waits[-1:], on_update=list(si.on_update)
            )
            for w in waits[:-1]:
                nop = mybir.InstNoOp(
                    name=self.nc.get_next_instruction_name(),
                    sync_info=mybir.SyncInfo(on_wait=[w], on_update=[]),
                    bass_nofuse=True,
                    engine=inst.engine,
                )
                orig_commit(self, nop, lazy_reg_writes=False)
        orig_commit(self, inst, lazy_reg_writes)

    def _patched(self, tick_clock, wait_clock):
        nc = self.nc
        drain_inst = nc.sync.drain()
        wait_clock.add_sem_waits(
            drain_inst.ins, ScopedClock({None: tick_clock.global_clock})
        )
        si = drain_inst.ins.sync_info
        if si is not None and len(si.on_wait) > 1:
            waits = list(si.on_wait)
            drain_inst.ins.sync_info = mybir.SyncInfo(
                on_wait=waits[:1], on_update=list(si.on_update)
            )
            for i in range(1, len(waits)):
                extra = nc.sync.drain()
                extra.ins.sync_info = mybir.SyncInfo(
                    on_wait=waits[i : i + 1], on_update=[]
                )
        nc.all_engine_barrier()
        assert self.sems is not None
        popped = nc._tile_sem_poison_stack.pop()
        assert popped is self._sem_poison
        nc.clear_and_free_semaphores(list(self.sems.allocated().values()))
        nc.all_engine_barrier()

    tile.TileContext._commit_instruction = _split_commit
    tile.TileContext._drain_and_barrier = _patched
    tile.TileContext._drain_patch_installed = True


def build_nc():
    _install_drain_patch()
    nc = bass.Bass()

    # per-core data
    # x / xq carry a host-appended ones row (row 64) for the bias-row trick
    x_d = nc.dram_tensor("x", [C + 1, N], F32, kind="ExternalInput")
    xq_d = nc.dram_tensor("xq", [C + 1, NQ], F32, kind="ExternalInput")
    xt_d = nc.dram_tensor("xt", [NQ, C], F32, kind="ExternalInput")
    # zeros block; straight DMA copies zero-fill the padded partition
    # ranges without touching the DVE critical path (few, large,
    # contiguous descriptors — broadcast APs exploded into 4k tiny ones)
    zp_d = nc.dram_tensor("zpad", [128, N], BF16, kind="ExternalInput")
    # replicated weights / constants
    wqk_d = nc.dram_tensor("w_qk", [C, 2 * C], BF16, kind="ExternalInput")
    wv_d = nc.dram_tensor("wv_t", [C, C], BF16, kind="ExternalInput")
    waug_d = nc.dram_tensor("w_aug", [C + 1, C + 1], BF16, kind="ExternalInput")
    # gbias columns: 0 gamma, 1 beta, 2 [bq;bk] stacked, 3 bv
    gb_d = nc.dram_tensor("gbias", [2 * C, 4], F32, kind="ExternalInput")
    brow_d = nc.dram_tensor("brows", [1, 3 * C], F32, kind="ExternalInput")
    bo_d = nc.dram_tensor("bo_bc", [128, C], F32, kind="ExternalInput")
    y_d = nc.dram_tensor("y", [NQ, C], F32, kind="ExternalOutput")

    with tile.TileContext(nc) as tc:
        with (
            tc.tile_pool(name="const", bufs=1) as const,
            tc.tile_pool(name="big", bufs=1) as big,
            tc.tile_pool(name="stats", bufs=2) as stats,
            tc.tile_pool(name="pt", bufs=4) as ptp,
            tc.tile_pool(name="tail", bufs=2) as tailp,
            tc.tile_pool(name="yp", bufs=3) as yp,
            tc.tile_pool(name="xtp", bufs=3) as xtp,
            tc.tile_pool(name="sps", bufs=2, space="PSUM") as sps,
            tc.tile_pool(name="ops", bufs=4, space="PSUM") as ops,
        ):
            # ---- load constants
            wqk = const.tile([C, 2 * C], BF16, tag="wqk")
            wv = const.tile([C, C], BF16, tag="wv")
            waug = const.tile([128, 128], BF16, tag="waug")
            gb = const.tile([2 * C, 4], F32, tag="gb")
            brow = const.tile([1, 3 * C], F32, tag="brow")
            bo_bc = const.tile([128, C], F32, tag="bo")
            gamma = gb[:C, 0:1]
            beta = gb[:C, 1:2]
            bqk_col = gb[:, 2:3]
            bv_col = gb[:C, 3:4]

            # ---- PE clock warmup: the PE clock ramps 1.2->2.4 GHz only
            # after sustained uninterrupted activity and then stays warm as
            # long as it never idles >~3us. Keep the PE busy with dummy
            # same-weight matmuls from kernel start until the projections are
            # ready (more reps are interleaved into the projection phase).
            warm_sb = const.tile([128, 512], BF16, tag="warm")
            nc.vector.memset(warm_sb, 0.0)

            def warm_reps(n, base):
                for i in range(n):
                    wp = ops.tile([128, 512], F32, tag="o", name=f"w{base}_{i}")
                    nc.tensor.matmul(
                        out=wp, lhsT=warm_sb[:, 0:128], rhs=warm_sb,
                        start=True, stop=True,
                    )

            warm_reps(WARMUP_REPS, "a")

            # chain-independent prep, off the GroupNorm critical path
            eps_t = stats.tile([C, 1], F32, tag="eps")
            nc.vector.memset(eps_t, EPS)
            # first ACT instruction = tiny Exp: walrus attaches the one-time
            # ~2.7us ACT_TABLE_LOAD here, in the startup dead zone, instead
            # of stalling the first real activation mid-kernel
            tblw = stats.tile([C, 1], F32, tag="tblw")
            nc.scalar.activation(out=tblw, in_=eps_t, func=AF.Exp)
            tp1 = stats.tile([C, 32], F32, tag="tp1")
            nc.vector.memset(tp1, 0.0)
            g_inv = stats.tile([C, 1], F32, tag="ginv")
            nc.vector.memset(waug, 0.0)

            # ---- load x; bn_stats per 512-chunk as chunks arrive.
            # bf16 copies of x/xq for the projection matmuls come via
            # gpsimd cast-DMAs (GroupNorm scale/shift are folded into the
            # projection weights, so no separate normalization pass).
            x_sb = big.tile([C + 1, N], F32, tag="x")
            xq_sb = big.tile([C + 1, NQ], F32, tag="xq")
            # projection inputs padded to 128 partitions (rows 65:128 zero)
            # so every matmul enables the full 128x128 array: the HAM clock
            # gate integrates the ENABLED array fraction, and half-array
            # matmuls (contract=64 / 65 out rows) get throttled to 1.2 GHz
            # even when the PE has zero idle gaps.
            x_bf = big.tile([128, N], BF16, tag="xbf")
            xq_bf = big.tile([128, NQ], BF16, tag="xqbf")

            # pads FIRST (row 64 = ones row is rewritten by the casts below,
            # which the tile tracker serializes after these DMAs)
            nc.sync.dma_start(out=x_bf[C:128, :], in_=zp_d[0:C, :])
            nc.sync.dma_start(out=xq_bf[C:128, :], in_=zp_d[0:C, 0:NQ])
            # issue ALL input-chunk DMAs before any dependent copy so no
            # dma_start sits behind a blocked copy in an engine FIFO
            for j in range(8):
                sl = bass.ts(j, 512)
                eng = nc.sync if j % 2 else nc.scalar
                eng.dma_start(out=x_sb[:, sl], in_=x_d[:, sl])
            for j in range(4):
                sl = bass.ts(j, 512)
                eng = nc.scalar if j % 2 else nc.sync
                eng.dma_start(out=xq_sb[:, sl], in_=xq_d[:, sl])
            st = stats.tile([C, 8, 6], F32, tag="bnst")
            for j in range(8):
                sl = bass.ts(j, 512)
                nc.vector.bn_stats(out=st[:, j, :], in_=x_sb[0:C, sl])
                # bf16 cast for the projection inputs on ScalarE (idle
                # here; gpsimd casts are ~3x slower and stall concurrent
                # DVE ops via port sharing)
                nc.scalar.copy(out=x_bf[0 : C + 1, sl], in_=x_sb[:, sl])
            for j in range(4):
                sl = bass.ts(j, 512)
                nc.scalar.copy(out=xq_bf[0 : C + 1, sl], in_=xq_sb[:, sl])
            nc.gpsimd.dma_start(out=wqk, in_=wqk_d[:, :])
            nc.gpsimd.dma_start(out=wv, in_=wv_d[:, :])
            nc.gpsimd.dma_start(out=waug[0 : C + 1, 0 : C + 1], in_=waug_d[:, :])
            nc.gpsimd.dma_start(out=gb, in_=gb_d[:, :])
            nc.gpsimd.dma_start(out=brow, in_=brow_d[:, :])
            nc.gpsimd.dma_start(out=bo_bc, in_=bo_d[:, :])
            mv = stats.tile([C, 2], F32, tag="mv")
            nc.vector.bn_aggr(out=mv, in_=st)
            # me2 = [mean, var + mean^2] per channel
            me2 = stats.tile([C, 2], F32, tag="me2")
            nc.vector.tensor_copy(out=me2[:, 0:1], in_=mv[:, 0:1])
            m2 = stats.tile([C, 1], F32, tag="m2")
            nc.vector.tensor_mul(out=m2, in0=mv[:, 0:1], in1=mv[:, 0:1])
            nc.vector.tensor_add(out=me2[:, 1:2], in0=mv[:, 1:2], in1=m2)
            # group (channel-pair) sums of [mean, E[x^2]] without touching
            # the PE: 32x32 stream-transpose, add adjacent columns, replicate,
            # transpose back.
            nc.vector.tensor_copy(out=tp1[:, 0:2], in_=me2)
            tp2 = stats.tile([C, 32], F32, tag="tp2")
            nc.vector.transpose(out=tp2, in_=tp1)
            t2v = tp2.rearrange("p (g two) -> p g two", two=2)
            tp3 = stats.tile([C, 16], F32, tag="tp3")
            nc.vector.tensor_add(out=tp3, in0=t2v[:, :, 0], in1=t2v[:, :, 1])
            tp4 = stats.tile([C, 32], F32, tag="tp4")
            t4v = tp4.rearrange("p (g two) -> p g two", two=2)
            nc.vector.tensor_copy(out=t4v[:, :, 0], in_=tp3)
            nc.vector.tensor_copy(out=t4v[:, :, 1], in_=tp3)
            tp5 = stats.tile([C, 32], F32, tag="tp5")
            nc.vector.transpose(out=tp5, in_=tp4)
            # tp5[:, 0] = 2*mean_g, tp5[:, 1] = 2*E[x^2]_g per channel
            mean_g = stats.tile([C, 1], F32, tag="meang")
            nc.vector.tensor_scalar(
                out=mean_g, in0=tp5[:, 0:1], scalar1=0.5, scalar2=None,
                op0=ALU.mult,
            )
            varg = stats.tile([C, 1], F32, tag="varg")
            nc.vector.tensor_mul(out=varg, in0=mean_g, in1=mean_g)
            nc.vector.scalar_tensor_tensor(
                out=varg, in0=tp5[:, 1:2], scalar=0.5, in1=varg,
                op0=ALU.mult, op1=ALU.subtract,
            )
            # rstd = 1/sqrt(var+eps);  s = rstd*gamma;  t = beta - mean*s
            nc.scalar.activation(out=varg, in_=varg, func=AF.Sqrt, bias=eps_t)

            rstd = stats.tile([C, 1], F32, tag="rstd")
            nc.vector.reciprocal(out=rstd, in_=varg)
            s_col = stats.tile([C, 1], F32, tag="scol")
            nc.vector.tensor_mul(out=s_col, in0=rstd, in1=gamma)
            t_col = stats.tile([C, 1], F32, tag="tcol")
            nc.vector.tensor_mul(out=t_col, in0=mean_g, in1=s_col)
            nc.vector.tensor_tensor(out=t_col, in0=beta, in1=t_col, op=ALU.subtract)

            # fold the GroupNorm affine into the projections:
            #   W @ (x*s + t) = (W*diag(s)) @ x + (W @ t)
            # the W@t bias goes in as a 65th contraction row against the
            # ones-row in x_bf/xq_bf, so projection copies are bias-free
            wqk_s = const.tile([128, 2 * C], BF16, tag="wqks")
            nc.vector.memset(wqk_s[C:128, :], 0.0)
            nc.vector.tensor_scalar_mul(out=wqk_s[0:C, :], in0=wqk, scalar1=s_col)
            wv_s = const.tile([128, 128], BF16, tag="wvs")
            nc.vector.memset(wv_s, 0.0)
            nc.vector.tensor_scalar_mul(
                out=wv_s[0:C, 0:C], in0=wv, scalar1=s_col
            )
            # bias matmuls reuse the SCALED stationaries: W@t = (W*s)@(t/s)
            # t/s = t * std / gamma   (varg holds sqrt(var+eps) here)
            nc.vector.reciprocal(out=g_inv, in_=gamma)
            s_inv = stats.tile([C, 1], F32, tag="sinv")
            nc.vector.tensor_mul(out=s_inv, in0=varg, in1=g_inv)
            t_bf = stats.tile([C, 1], BF16, tag="tbf")
            nc.vector.tensor_mul(out=t_bf, in0=t_col, in1=s_inv)

            # ---- QK fused pass over xn (rows 0:64 = Q+bq, 64:128 = K+bk),
            #      V pass, all with one stationary each
            # K and Q padded to 128 partitions (zeros below row 64) so the
            # score matmuls contract over the full 128 array rows
            k_sb = big.tile([128, N], BF16, tag="k")
            q_sb = big.tile([128, NQ], BF16, tag="q")
            v_sb = big.tile([C, N], BF16, tag="v")
            nc.sync.dma_start(out=k_sb[C:128, :], in_=zp_d[0:C, :])
            nc.sync.dma_start(out=q_sb[C:128, :], in_=zp_d[0:C, 0:NQ])
            # bias rows: (t/s)^T @ (W*s) + b  -> row 64 of each stationary
            trow_ps = sps.tile([1, 2 * C], F32, tag="sps", name="trowqk")
            nc.tensor.matmul(
                out=trow_ps, lhsT=t_bf, rhs=wqk_s[0:C, :], start=True, stop=True
            )
            nc.vector.tensor_add(
                out=wqk_s[C : C + 1, :], in0=trow_ps, in1=brow[0:1, 0 : 2 * C]
            )
            tvow_ps = sps.tile([1, C], F32, tag="sps", name="trowv")
            nc.tensor.matmul(
                out=tvow_ps, lhsT=t_bf, rhs=wv_s[0:C, 0:C], start=True, stop=True
            )
            nc.vector.tensor_add(
                out=wv_s[C : C + 1, 0:C], in0=tvow_ps,
                in1=brow[0:1, 2 * C : 3 * C],
            )
            # one wqk_s-stationary run: 8 chunks of x (K rows) + 4 of xq (Q),
            # pure copies strictly alternating ACT/DVE so the PE never stalls
            for j in range(8):
                sl = bass.ts(j, 512)
                ps = ops.tile([128, 512], F32, tag="o", name=f"qk{j}")
                nc.tensor.matmul(
                    out=ps, lhsT=wqk_s, rhs=x_bf[:, sl], start=True, stop=True
                )
                if j % 2:
                    nc.scalar.activation(
                        out=k_sb[0:C, sl], in_=ps[64:128, :], func=AF.Copy
                    )
                else:
                    nc.vector.tensor_copy(out=k_sb[0:C, sl], in_=ps[64:128, :])
            for j in range(4):
                sl = bass.ts(j, 512)
                ps = ops.tile([128, 512], F32, tag="o", name=f"qq{j}")
                nc.tensor.matmul(
                    out=ps, lhsT=wqk_s, rhs=xq_bf[:, sl], start=True, stop=True
                )
                if j % 2:
                    nc.scalar.activation(
                        out=q_sb[0:C, sl], in_=ps[0:64, :], func=AF.Copy
                    )
                else:
                    nc.vector.tensor_copy(out=q_sb[0:C, sl], in_=ps[0:64, :])
            # then one wv_s-stationary run; V^T xbar transposes per pair so
            # the first key blocks are ready as the attention loop starts
            for j in range(8):
                sl = bass.ts(j, 512)
                ps = ops.tile([128, 512], F32, tag="o", name=f"v{j}")
                nc.tensor.matmul(
                    out=ps, lhsT=wv_s, rhs=x_bf[:, sl], start=True, stop=True
                )
                if j % 2:
                    nc.scalar.activation(
                        out=v_sb[:, sl], in_=ps[0:C, :], func=AF.Copy
                    )
                else:
                    nc.vector.tensor_copy(out=v_sb[:, sl], in_=ps[0:C, :])

            # ---- V^T blocks [128, 65] with ones column, via DMA xbar
            # per-block stride padded to 128 elements: the xbar transpose
            # needs 128B-aligned destination offsets. out[p, kb, c] =
            # V^T[kb*128 + p, c]; 4 chunked calls so early key blocks are
            # ready as soon as their v chunks are copied.
            vt = big.tile([128, KB, 128], BF16, tag="vt")
            # zero the whole vt tile (contiguous, 128 descriptors); the
            # transposes and the ones column overwrite their regions after.
            # Cols C+1:128 stay zero so PV matmuls enable all four 32-col
            # groups of the array (out rows 65:128 accumulate zeros)
            nc.sync.dma_start(
                out=vt,
                in_=zp_d[:, :].rearrange("p (a b) -> p a b", b=128),
            )
            for t in range(4):
                nc.sync.dma_start_transpose(
                    out=vt[:, bass.ds(t * 8, 8), 0:C],
                    in_=v_sb[:, bass.ts(t, 1024)],
                )
            nc.vector.memset(vt[:, :, C : C + 1], 1.0)

            # ---- whole-xt load up front (tail residual input)
            xt_all = xtp.tile([128, 16, C], F32, tag="xt", bufs=1)
            nc.sync.dma_start(
                out=xt_all, in_=xt_d.rearrange("(j p) c -> p j c", p=128)
            )

            # ---- main attention loop
            o_tiles = [
                ops.tile([128, 512], F32, tag="o", name=f"o{qc}")
                for qc in range(4)
            ]
            # software-pipelined: PV for block kb-1 is emitted after the score
            # matmuls for block kb, so the PE does 4 same-stationary matmuls
            # per weight switch and exp(kb-1) has a full block to finish.
            def emit_pv(kb, p2, after):
                for qc in range(4):
                    mm = nc.tensor.matmul(
                        out=o_tiles[qc], lhsT=vt[:, kb, 0:128],
                        rhs=p2[qc // 2][:, (qc % 2) * 512 : (qc % 2 + 1) * 512],
                        start=(kb == 0), stop=(kb == KB - 1),
                        skip_group_check=True,
                    )
                    if qc == 0 and after is not None:
                        # keep the PE stream in same-stationary runs of 4:
                        # PV(kb-1) only after the last score matmul of kb
                        add_dep_helper(
                            mm.ins, after.ins, sync=False,
                            reason="group PE same-stationary runs",
                        )

            prev = None
            for kb in range(KB):
                kblk = k_sb[:, bass.ts(kb, 128)]
                s2 = []
                last_s = None
                for h in range(2):
                    sp = sps.tile([128, 1024], F32, tag="sps", name=f"s{kb}_{h}")
                    nc.tensor.matmul(
                        out=sp[:, 0:512], lhsT=kblk,
                        rhs=q_sb[:, bass.ds(h * 1024, 512)],
                        start=True, stop=True,
                    )
                    last_s = nc.tensor.matmul(
                        out=sp[:, 512:1024], lhsT=kblk,
                        rhs=q_sb[:, bass.ds(h * 1024 + 512, 512)],
                        start=True, stop=True,
                    )
                    s2.append(sp)
                p2 = []
                for h in range(2):
                    p = ptp.tile([128, 1024], BF16, tag="p", name=f"p{kb}_{h}")
                    nc.scalar.activation(out=p, in_=s2[h], func=AF.Exp, scale=0.125)
                    p2.append(p)
                if kb in (1, 2):
                    # pad the PE over exp(0)'s table load + latency so the
                    # pipeline fill doesn't leave a clock-dropping idle gap
                    warm_reps(3, f"fill{kb}")
                if prev is not None:
                    emit_pv(kb - 1, prev, last_s)
                prev = p2
            emit_pv(KB - 1, prev, None)

            # ---- tail: project through augmented Wo, DMA-transpose,
            #      normalize by denominator, add residual + bo, store
            # xt was loaded up front (xt_all); add bo once, broadcast over j
            xtb = xtp.tile([128, 16, C], F32, tag="xtb", bufs=1)
            bo_bcast = bass.AP(
                tensor=bo_bc.tensor, offset=bo_bc.offset,
                ap=[list(bo_bc.ap[0]), [0, 16], list(bo_bc.ap[1])],
            )
            nc.vector.tensor_add(out=xtb, in0=xt_all, in1=bo_bcast)

            # tail in two halves so transpose/normalize/store of half 0
            # overlap the z-projection of half 1
            z_all = tailp.tile([80, 2048], BF16, tag="z", bufs=1)
            zt_all = tailp.tile([128, 16, 128], BF16, tag="zt", bufs=1)
            r_all = yp.tile([128, 16], F32, tag="r", bufs=1)
            y_all = yp.tile([128, 16, C], F32, tag="y", bufs=1)
            y_view = y_d.rearrange("(j p) c -> p j c", p=128)
            for hh in range(2):
                for qc in (2 * hh, 2 * hh + 1):
                    ou = tailp.tile([128, 512], BF16, tag="ou")
                    if qc % 2:
                        nc.vector.tensor_copy(out=ou, in_=o_tiles[qc])
                    else:
                        nc.scalar.activation(
                            out=ou, in_=o_tiles[qc], func=AF.Copy
                        )
                    z_ps = sps.tile([128, 512], F32, tag="sps", name=f"z{qc}")
                    nc.tensor.matmul(
                        out=z_ps, lhsT=waug, rhs=ou, start=True, stop=True
                    )
                    if qc % 2:
                        nc.scalar.activation(
                            out=z_all[0 : C + 1, bass.ts(qc, 512)],
                            in_=z_ps[0 : C + 1, :],
                            func=AF.Copy,
                        )
                    else:
                        nc.vector.tensor_copy(
                            out=z_all[0 : C + 1, bass.ts(qc, 512)],
                            in_=z_ps[0 : C + 1, :],
                        )
                heng = nc.scalar if hh else nc.sync
                jsl = bass.ds(hh * 8, 8)
                heng.dma_start_transpose(
                    out=zt_all[:, jsl, 0:80],
                    in_=z_all[:, bass.ds(hh * 1024, 1024)],
                )
                nc.vector.reciprocal(
                    out=r_all[:, jsl], in_=zt_all[:, jsl, C]
                )
                for j in range(8 * hh, 8 * hh + 8):
                    nc.vector.scalar_tensor_tensor(
                        out=y_all[:, j, :], in0=zt_all[:, j, 0:C],
                        scalar=r_all[:, j : j + 1], in1=xtb[:, j, :],
                        op0=ALU.mult, op1=ALU.add,
                    )
                heng.dma_start(out=y_view[:, jsl, :], in_=y_all[:, jsl, :])
    return nc


_NC = None


def _get_nc():
    global _NC
    if _NC is None:
        _NC = build_nc()
    return _NC


def _prep_maps(x, Wq, bq, Wk, bk, Wv, bv, Wo, bo, gamma, beta):
    bf = ml_dtypes.bfloat16
    w_qk = np.concatenate([Wq.T, Wk.T], axis=1).astype(bf)  # [cin, 2c]
    wv_t = np.ascontiguousarray(Wv.T).astype(bf)
    w_aug = np.zeros((C + 1, C + 1), np.float32)
    w_aug[:C, :C] = Wo.T
    w_aug[C, C] = 1.0
    w_aug = w_aug.astype(bf)
    gbias = np.zeros((2 * C, 4), np.float32)
    gbias[:C, 0] = gamma
    gbias[:C, 1] = beta
    gbias[:, 2] = np.concatenate([bq, bk])
    gbias[:C, 3] = bv
    brows = np.zeros((1, 3 * C), np.float32)
    brows[0, : 2 * C] = np.concatenate([bq, bk])
    brows[0, 2 * C :] = bv
    bo_bc = np.tile(bo[None, :], (128, 1)).astype(np.float32)

    shared = dict(
        w_qk=w_qk, wv_t=wv_t, w_aug=w_aug, gbias=gbias, brows=brows,
        bo_bc=bo_bc, zpad=np.zeros((128, N), bf),
    )
    in_maps = []
    for core in range(8):
        b, half = core // 2, core % 2
        xm = np.ascontiguousarray(x[b].reshape(C, N)).astype(np.float32)
        ones = np.ones((1, N), np.float32)
        xm1 = np.concatenate([xm, ones], axis=0)
        xqm = np.ascontiguousarray(xm1[:, half * NQ : (half + 1) * NQ])
        xtm = np.ascontiguousarray(xm.T[half * NQ : (half + 1) * NQ, :])
        in_maps.append(dict(shared, x=xm1, xq=xqm, xt=xtm))
    return in_maps


def run(inputs, trace=False):
    from concourse.bass_utils import run_bass_kernel_spmd

    inputs = {k: np.asarray(v) for k, v in inputs.items()}
    nc = _get_nc()
    in_maps = _prep_maps(**inputs)
    res = run_bass_kernel_spmd(
        nc, in_maps, core_ids=list(range(8)), trace=trace
    )
    out = np.empty((B, C, N), np.float32)
    for core in range(8):
        b, half = core // 2, core % 2
        out[b][:, half * NQ : (half + 1) * NQ] = res.results[core]["y"].T
    return out.reshape(B, C, H, W), res


def kernel(**inputs):
    out, _ = run(inputs, trace=False)
    return out



# revision 36
# speedup vs baseline: 1.4286x; 1.0079x over previous
"""Trainium2 Bass kernel for the AttentionBlock problem.

Fixed problem shape: x [4, 64, 64, 64] fp32, GroupNorm(32 groups) ->
1x1 conv Q/K/V -> softmax(Q^T K / 8) -> V @ attn^T -> 1x1 conv + residual.

Sharding: 8 cores, core = 2*batch + query_half. Each core holds its batch's
full x (for K/V) and computes outputs for its 2048-query half.

Layout strategy (per core):
  - x, K, Q, V live as [c=64 partitions, n free]; matmul operands in bf16
    with fp32 PSUM accumulation (attention is ~5% of the residual output,
    so final norm-rel-err stays ~1e-3).
  - GroupNorm stats via bn_stats + a DVE 32x32 stream-transpose pair-combine
    (PE-free); the affine fold goes INTO the projection weights
    (W*diag(s) stationaries) and a bias row (t/s)^T@(W*s)+b against a
    host-appended ones-row of x, so there is no normalization pass at all.
  - Scores are computed TRANSPOSED: S_T[k,q] = K_blk^T Q (contract c on
    partitions); the softmax denominator comes free from a ones-column
    appended to V^T during the PV matmul (no cross-partition reductions).
  - exp() runs on ScalarE directly out of PSUM in 1024-wide ops, no max
    subtraction (scores are O(+-10) here; exp stays well inside fp32 range).
  - PE stationary operands switch only twice per key block (K_blk for 4
    score matmuls, V^T_blk for 4 PV matmuls); PV for block kb-1 is emitted
    after the scores of kb (software pipeline).
  - V^T blocks and the output-projection transposes go through the DMA xbar
    transpose (bf16, 128B-aligned dst offsets), keeping them off the PE.
  - The PE clock on this part ramps 1.2->2.4 GHz only after sustained
    uninterrupted activity and re-throttles after idle gaps; a dummy
    same-weight warmup burst keeps the PE busy from kernel start through
    the projections so the attention loop can run warm (~125 us) instead
    of cold (~177 us) when the chip's power state cooperates.
  - Tail: augmented 65x65 Wo carries the denominators through the output
    projection; one xbar transpose puts q on partitions, one batched
    reciprocal + 16 fused multiply-adds apply 1/denom + residual + bias,
    one strided DMA writes y back.
"""

import numpy as np
import ml_dtypes

import concourse.bass as bass
import concourse.mybir as mybir
import concourse.tile as tile
from concourse.tile_rust import add_dep_helper
from concourse.vector_clock import ScopedClock

B, C, H, W = 4, 64, 64, 64
N = H * W            # 4096
NQ = N // 2          # queries per core
EPS = 1e-5
KB = 32              # key blocks of 128
WARMUP_REPS = 26     # initial PE warmup burst (more interleaved later)
F32 = mybir.dt.float32
BF16 = mybir.dt.bfloat16
AF = mybir.ActivationFunctionType
ALU = mybir.AluOpType


# ---------------------------------------------------------------------------
# This container's walrus codegen rejects >1 sync wait on one instruction
# ("Too many sync wait commands") — split extra waits onto preceding same-
# engine NOPs (engines execute in order, so semantics are preserved), and do
# the same for the TileContext tail drain.
def _install_drain_patch():
    if getattr(tile.TileContext, "_drain_patch_installed", False):
        return

    orig_commit = tile.TileContext._commit_instruction

    def _split_commit(self, inst, lazy_reg_writes=True):
        si = getattr(inst, "sync_info", None)
        if (
            si is not None
            and len(si.on_wait) > 1
            and inst.engine != mybir.EngineType.Unassigned
        ):
            waits = list(si.on_wait)
            inst.sync_info = mybir.SyncInfo(
                on_wait=waits[-1:], on_update=list(si.on_update)
            )
            for w in waits[:-1]:
                nop = mybir.InstNoOp(
                    name=self.nc.get_next_instruction_name(),
                    sync_info=mybir.SyncInfo(on_wait=[w], on_update=[]),
                    bass_nofuse=True,
                    engine=inst.engine,
                )
                orig_commit(self, nop, lazy_reg_writes=False)
        orig_commit(self, inst, lazy_reg_writes)

    def _patched(self, tick_clock, wait_clock):
        nc = self.nc
        drain_inst = nc.sync.drain()
        wait_clock.add_sem_waits(
            drain_inst.ins, ScopedClock({None: tick_clock.global_clock})
        )
        si = drain_inst.ins.sync_info
        if si is not None and len(si.on_wait) > 1:
            waits = list(si.on_wait)
            drain_inst.ins.sync_info = mybir.SyncInfo(
                on_wait=waits[:1], on_update=list(si.on_update)
            )
            for i in range(1, len(waits)):
                extra = nc.sync.drain()
                extra.ins.sync_info = mybir.SyncInfo(
                    on_wait=waits[i : i + 1], on_update=[]
                )
        nc.all_engine_barrier()
        assert self.sems is not None
        popped = nc._tile_sem_poison_stack.pop()
        assert popped is self._sem_poison
        nc.clear_and_free_semaphores(list(self.sems.allocated().values()))
        nc.all_engine_barrier()

    tile.TileContext._commit_instruction = _split_commit
    tile.TileContext._drain_and_barrier = _patched
    tile.TileContext._drain_patch_installed = True


def build_nc():
    _install_drain_patch()
    nc = bass.Bass()

    # per-core data
    # x / xq carry a host-appended ones row (row 64) for the bias-row trick
    x_d = nc.dram_tensor("x", [C + 1, N], F32, kind="ExternalInput")
    xq_d = nc.dram_tensor("xq", [C + 1, NQ], F32, kind="ExternalInput")
    xt_d = nc.dram_tensor("xt", [NQ, C], F32, kind="ExternalInput")
    # replicated weights / constants
    wqk_d = nc.dram_tensor("w_qk", [C, 2 * C], BF16, kind="ExternalInput")
    wv_d = nc.dram_tensor("wv_t", [C, C], BF16, kind="ExternalInput")
    waug_d = nc.dram_tensor("w_aug", [C + 1, C + 1], BF16, kind="ExternalInput")
    # gbias columns: 0 gamma, 1 beta, 2 [bq;bk] stacked, 3 bv
    gb_d = nc.dram_tensor("gbias", [2 * C, 4], F32, kind="ExternalInput")
    brow_d = nc.dram_tensor("brows", [1, 3 * C], F32, kind="ExternalInput")
    bo_d = nc.dram_tensor("bo_bc", [128, C], F32, kind="ExternalInput")
    y_d = nc.dram_tensor("y", [NQ, C], F32, kind="ExternalOutput")

    with tile.TileContext(nc) as tc:
        with (
            tc.tile_pool(name="const", bufs=1) as const,
            tc.tile_pool(name="big", bufs=1) as big,
            tc.tile_pool(name="stats", bufs=2) as stats,
            tc.tile_pool(name="pt", bufs=4) as ptp,
            tc.tile_pool(name="tail", bufs=2) as tailp,
            tc.tile_pool(name="yp", bufs=3) as yp,
            tc.tile_pool(name="xtp", bufs=3) as xtp,
            tc.tile_pool(name="sps", bufs=2, space="PSUM") as sps,
            tc.tile_pool(name="ops", bufs=4, space="PSUM") as ops,
        ):
            # ---- load constants
            wqk = const.tile([C, 2 * C], BF16, tag="wqk")
            wv = const.tile([C, C], BF16, tag="wv")
            waug = const.tile([128, 128], BF16, tag="waug")
            gb = const.tile([2 * C, 4], F32, tag="gb")
            brow = const.tile([1, 3 * C], F32, tag="brow")
            bo_bc = const.tile([128, C], F32, tag="bo")
            gamma = gb[:C, 0:1]
            beta = gb[:C, 1:2]
            bqk_col = gb[:, 2:3]
            bv_col = gb[:C, 3:4]

            # ---- PE clock warmup: the PE clock ramps 1.2->2.4 GHz only
            # after sustained uninterrupted activity and then stays warm as
            # long as it never idles >~3us. Keep the PE busy with dummy
            # same-weight matmuls from kernel start until the projections are
            # ready (more reps are interleaved into the projection phase).
            warm_sb = const.tile([128, 512], BF16, tag="warm")
            nc.vector.memset(warm_sb, 0.0)

            def warm_reps(n, base):
                for i in range(n):
                    wp = ops.tile([128, 512], F32, tag="o", name=f"w{base}_{i}")
                    nc.tensor.matmul(
                        out=wp, lhsT=warm_sb[:, 0:128], rhs=warm_sb,
                        start=True, stop=True,
                    )

            warm_reps(WARMUP_REPS, "a")

            # chain-independent prep, off the GroupNorm critical path
            eps_t = stats.tile([C, 1], F32, tag="eps")
            nc.vector.memset(eps_t, EPS)
            # first ACT instruction = tiny Exp: walrus attaches the one-time
            # ~2.7us ACT_TABLE_LOAD here, in the startup dead zone, instead
            # of stalling the first real activation mid-kernel
            tblw = stats.tile([C, 1], F32, tag="tblw")
            nc.scalar.activation(out=tblw, in_=eps_t, func=AF.Exp)
            tp1 = stats.tile([C, 32], F32, tag="tp1")
            nc.vector.memset(tp1, 0.0)
            g_inv = stats.tile([C, 1], F32, tag="ginv")
            nc.vector.memset(waug, 0.0)

            # ---- load x; bn_stats per 512-chunk as chunks arrive.
            # bf16 copies of x/xq for the projection matmuls come via
            # gpsimd cast-DMAs (GroupNorm scale/shift are folded into the
            # projection weights, so no separate normalization pass).
            x_sb = big.tile([C + 1, N], F32, tag="x")
            xq_sb = big.tile([C + 1, NQ], F32, tag="xq")
            # projection inputs padded to 128 partitions (rows 65:128 zero)
            # so every matmul enables the full 128x128 array: the HAM clock
            # gate integrates the ENABLED array fraction, and half-array
            # matmuls (contract=64 / 65 out rows) get throttled to 1.2 GHz
            # even when the PE has zero idle gaps.
            x_bf = big.tile([128, N], BF16, tag="xbf")
            xq_bf = big.tile([128, NQ], BF16, tag="xqbf")

            # pads FIRST (row 64 = ones row is rewritten by the casts below,
            # which the tile tracker serializes after these DMAs)
            nc.gpsimd.memset(x_bf[C:128, :], 0.0)
            nc.gpsimd.memset(xq_bf[C:128, :], 0.0)
            # issue ALL input-chunk DMAs before any dependent copy so no
            # dma_start sits behind a blocked copy in an engine FIFO
            for j in range(8):
                sl = bass.ts(j, 512)
                eng = nc.sync if j % 2 else nc.scalar
                eng.dma_start(out=x_sb[:, sl], in_=x_d[:, sl])
            for j in range(4):
                sl = bass.ts(j, 512)
                eng = nc.scalar if j % 2 else nc.sync
                eng.dma_start(out=xq_sb[:, sl], in_=xq_d[:, sl])
            st = stats.tile([C, 8, 6], F32, tag="bnst")
            for j in range(8):
                sl = bass.ts(j, 512)
                nc.vector.bn_stats(out=st[:, j, :], in_=x_sb[0:C, sl])
                # bf16 cast for the projection inputs on ScalarE (idle
                # here; gpsimd casts are ~3x slower and stall concurrent
                # DVE ops via port sharing)
                nc.scalar.copy(out=x_bf[0 : C + 1, sl], in_=x_sb[:, sl])
            for j in range(4):
                sl = bass.ts(j, 512)
                nc.scalar.copy(out=xq_bf[0 : C + 1, sl], in_=xq_sb[:, sl])
            nc.sync.dma_start(out=wqk, in_=wqk_d[:, :])
            nc.sync.dma_start(out=wv, in_=wv_d[:, :])
            nc.sync.dma_start(out=waug[0 : C + 1, 0 : C + 1], in_=waug_d[:, :])
            nc.sync.dma_start(out=gb, in_=gb_d[:, :])
            nc.sync.dma_start(out=brow, in_=brow_d[:, :])
            nc.sync.dma_start(out=bo_bc, in_=bo_d[:, :])
            mv = stats.tile([C, 2], F32, tag="mv")
            nc.vector.bn_aggr(out=mv, in_=st)
            # me2 = [mean, var + mean^2] per channel
            me2 = stats.tile([C, 2], F32, tag="me2")
            nc.vector.tensor_copy(out=me2[:, 0:1], in_=mv[:, 0:1])
            m2 = stats.tile([C, 1], F32, tag="m2")
            nc.vector.tensor_mul(out=m2, in0=mv[:, 0:1], in1=mv[:, 0:1])
            nc.vector.tensor_add(out=me2[:, 1:2], in0=mv[:, 1:2], in1=m2)
            # group (channel-pair) sums of [mean, E[x^2]] without touching
            # the PE: 32x32 stream-transpose, add adjacent columns, replicate,
            # transpose back.
            nc.vector.tensor_copy(out=tp1[:, 0:2], in_=me2)
            tp2 = stats.tile([C, 32], F32, tag="tp2")
            nc.vector.transpose(out=tp2, in_=tp1)
            t2v = tp2.rearrange("p (g two) -> p g two", two=2)
            tp3 = stats.tile([C, 16], F32, tag="tp3")
            nc.vector.tensor_add(out=tp3, in0=t2v[:, :, 0], in1=t2v[:, :, 1])
            tp4 = stats.tile([C, 32], F32, tag="tp4")
            t4v = tp4.rearrange("p (g two) -> p g two", two=2)
            nc.vector.tensor_copy(out=t4v[:, :, 0], in_=tp3)
            nc.vector.tensor_copy(out=t4v[:, :, 1], in_=tp3)
            tp5 = stats.tile([C, 32], F32, tag="tp5")
            nc.vector.transpose(out=tp5, in_=tp4)
            # tp5[:, 0] = 2*mean_g, tp5[:, 1] = 2*E[x^2]_g per channel
            mean_g = stats.tile([C, 1], F32, tag="meang")
            nc.vector.tensor_scalar(
                out=mean_g, in0=tp5[:, 0:1], scalar1=0.5, scalar2=None,
                op0=ALU.mult,
            )
            varg = stats.tile([C, 1], F32, tag="varg")
            nc.vector.tensor_mul(out=varg, in0=mean_g, in1=mean_g)
            nc.vector.scalar_tensor_tensor(
                out=varg, in0=tp5[:, 1:2], scalar=0.5, in1=varg,
                op0=ALU.mult, op1=ALU.subtract,
            )
            # rstd = 1/sqrt(var+eps);  s = rstd*gamma;  t = beta - mean*s
            nc.scalar.activation(out=varg, in_=varg, func=AF.Sqrt, bias=eps_t)

            rstd = stats.tile([C, 1], F32, tag="rstd")
            nc.vector.reciprocal(out=rstd, in_=varg)
            s_col = stats.tile([C, 1], F32, tag="scol")
            nc.vector.tensor_mul(out=s_col, in0=rstd, in1=gamma)
            t_col = stats.tile([C, 1], F32, tag="tcol")
            nc.vector.tensor_mul(out=t_col, in0=mean_g, in1=s_col)
            nc.vector.tensor_tensor(out=t_col, in0=beta, in1=t_col, op=ALU.subtract)

            # fold the GroupNorm affine into the projections:
            #   W @ (x*s + t) = (W*diag(s)) @ x + (W @ t)
            # the W@t bias goes in as a 65th contraction row against the
            # ones-row in x_bf/xq_bf, so projection copies are bias-free
            wqk_s = const.tile([128, 2 * C], BF16, tag="wqks")
            nc.vector.memset(wqk_s[C:128, :], 0.0)
            nc.vector.tensor_scalar_mul(out=wqk_s[0:C, :], in0=wqk, scalar1=s_col)
            wv_s = const.tile([128, 128], BF16, tag="wvs")
            nc.vector.memset(wv_s, 0.0)
            nc.vector.tensor_scalar_mul(
                out=wv_s[0:C, 0:C], in0=wv, scalar1=s_col
            )
            # bias matmuls reuse the SCALED stationaries: W@t = (W*s)@(t/s)
            # t/s = t * std / gamma   (varg holds sqrt(var+eps) here)
            nc.vector.reciprocal(out=g_inv, in_=gamma)
            s_inv = stats.tile([C, 1], F32, tag="sinv")
            nc.vector.tensor_mul(out=s_inv, in0=varg, in1=g_inv)
            t_bf = stats.tile([C, 1], BF16, tag="tbf")
            nc.vector.tensor_mul(out=t_bf, in0=t_col, in1=s_inv)

            # ---- QK fused pass over xn (rows 0:64 = Q+bq, 64:128 = K+bk),
            #      V pass, all with one stationary each
            # K and Q padded to 128 partitions (zeros below row 64) so the
            # score matmuls contract over the full 128 array rows
            k_sb = big.tile([128, N], BF16, tag="k")
            q_sb = big.tile([128, NQ], BF16, tag="q")
            v_sb = big.tile([C, N], BF16, tag="v")
            nc.gpsimd.memset(k_sb[C:128, :], 0.0)
            nc.gpsimd.memset(q_sb[C:128, :], 0.0)
            # bias rows: (t/s)^T @ (W*s) + b  -> row 64 of each stationary
            trow_ps = sps.tile([1, 2 * C], F32, tag="sps", name="trowqk")
            nc.tensor.matmul(
                out=trow_ps, lhsT=t_bf, rhs=wqk_s[0:C, :], start=True, stop=True
            )
            nc.vector.tensor_add(
                out=wqk_s[C : C + 1, :], in0=trow_ps, in1=brow[0:1, 0 : 2 * C]
            )
            tvow_ps = sps.tile([1, C], F32, tag="sps", name="trowv")
            nc.tensor.matmul(
                out=tvow_ps, lhsT=t_bf, rhs=wv_s[0:C, 0:C], start=True, stop=True
            )
            nc.vector.tensor_add(
                out=wv_s[C : C + 1, 0:C], in0=tvow_ps,
                in1=brow[0:1, 2 * C : 3 * C],
            )
            # one wqk_s-stationary run: 8 chunks of x (K rows) + 4 of xq (Q),
            # pure copies strictly alternating ACT/DVE so the PE never stalls
            for j in range(8):
                sl = bass.ts(j, 512)
                ps = ops.tile([128, 512], F32, tag="o", name=f"qk{j}")
                nc.tensor.matmul(
                    out=ps, lhsT=wqk_s, rhs=x_bf[:, sl], start=True, stop=True
                )
                if j % 2:
                    nc.scalar.activation(
                        out=k_sb[0:C, sl], in_=ps[64:128, :], func=AF.Copy
                    )
                else:
                    nc.vector.tensor_copy(out=k_sb[0:C, sl], in_=ps[64:128, :])
            for j in range(4):
                sl = bass.ts(j, 512)
                ps = ops.tile([128, 512], F32, tag="o", name=f"qq{j}")
                nc.tensor.matmul(
                    out=ps, lhsT=wqk_s, rhs=xq_bf[:, sl], start=True, stop=True
                )
                if j % 2:
                    nc.scalar.activation(
                        out=q_sb[0:C, sl], in_=ps[0:64, :], func=AF.Copy
                    )
                else:
                    nc.vector.tensor_copy(out=q_sb[0:C, sl], in_=ps[0:64, :])
            # then one wv_s-stationary run; V^T xbar transposes per pair so
            # the first key blocks are ready as the attention loop starts
            for j in range(8):
                sl = bass.ts(j, 512)
                ps = ops.tile([128, 512], F32, tag="o", name=f"v{j}")
                nc.tensor.matmul(
                    out=ps, lhsT=wv_s, rhs=x_bf[:, sl], start=True, stop=True
                )
                if j % 2:
                    nc.scalar.activation(
                        out=v_sb[:, sl], in_=ps[0:C, :], func=AF.Copy
                    )
                else:
                    nc.vector.tensor_copy(out=v_sb[:, sl], in_=ps[0:C, :])

            # ---- V^T blocks [128, 65] with ones column, via DMA xbar
            # per-block stride padded to 128 elements: the xbar transpose
            # needs 128B-aligned destination offsets. out[p, kb, c] =
            # V^T[kb*128 + p, c]; 4 chunked calls so early key blocks are
            # ready as soon as their v chunks are copied.
            vt = big.tile([128, KB, 128], BF16, tag="vt")
            # zero the whole vt tile (contiguous, 128 descriptors); the
            # transposes and the ones column overwrite their regions after.
            # Cols C+1:128 stay zero so PV matmuls enable all four 32-col
            # groups of the array (out rows 65:128 accumulate zeros)
            nc.gpsimd.memset(vt, 0.0)
            for t in range(4):
                nc.sync.dma_start_transpose(
                    out=vt[:, bass.ds(t * 8, 8), 0:C],
                    in_=v_sb[:, bass.ts(t, 1024)],
                )
            nc.vector.memset(vt[:, :, C : C + 1], 1.0)

            # ---- whole-xt load up front (tail residual input)
            xt_all = xtp.tile([128, 16, C], F32, tag="xt", bufs=1)
            nc.sync.dma_start(
                out=xt_all, in_=xt_d.rearrange("(j p) c -> p j c", p=128)
            )

            # ---- main attention loop
            o_tiles = [
                ops.tile([128, 512], F32, tag="o", name=f"o{qc}")
                for qc in range(4)
            ]
            # software-pipelined: PV for block kb-1 is emitted after the score
            # matmuls for block kb, so the PE does 4 same-stationary matmuls
            # per weight switch and exp(kb-1) has a full block to finish.
            def emit_pv(kb, p2, after):
                for qc in range(4):
                    mm = nc.tensor.matmul(
                        out=o_tiles[qc], lhsT=vt[:, kb, 0:128],
                        rhs=p2[qc // 2][:, (qc % 2) * 512 : (qc % 2 + 1) * 512],
                        start=(kb == 0), stop=(kb == KB - 1),
                        skip_group_check=True,
                    )
                    if qc == 0 and after is not None:
                        # keep the PE stream in same-stationary runs of 4:
                        # PV(kb-1) only after the last score matmul of kb
                        add_dep_helper(
                            mm.ins, after.ins, sync=False,
                            reason="group PE same-stationary runs",
                        )

            prev = None
            for kb in range(KB):
                kblk = k_sb[:, bass.ts(kb, 128)]
                s2 = []
                last_s = None
                for h in range(2):
                    sp = sps.tile([128, 1024], F32, tag="sps", name=f"s{kb}_{h}")
                    nc.tensor.matmul(
                        out=sp[:, 0:512], lhsT=kblk,
                        rhs=q_sb[:, bass.ds(h * 1024, 512)],
                        start=True, stop=True,
                    )
                    last_s = nc.tensor.matmul(
                        out=sp[:, 512:1024], lhsT=kblk,
                        rhs=q_sb[:, bass.ds(h * 1024 + 512, 512)],
                        start=True, stop=True,
                    )
                    s2.append(sp)
                p2 = []
                for h in range(2):
                    p = ptp.tile([128, 1024], BF16, tag="p", name=f"p{kb}_{h}")
                    nc.scalar.activation(out=p, in_=s2[h], func=AF.Exp, scale=0.125)
                    p2.append(p)
                if kb in (1, 2):
                    # pad the PE over exp(0)'s table load + latency so the
                    # pipeline fill doesn't leave a clock-dropping idle gap
                    warm_reps(3, f"fill{kb}")
                if prev is not None:
                    emit_pv(kb - 1, prev, last_s)
                prev = p2
            emit_pv(KB - 1, prev, None)

            # ---- tail: project through augmented Wo, DMA-transpose,
            #      normalize by denominator, add residual + bo, store
            # xt was loaded up front (xt_all); add bo once, broadcast over j
            xtb = xtp.tile([128, 16, C], F32, tag="xtb", bufs=1)
            bo_bcast = bass.AP(
                tensor=bo_bc.tensor, offset=bo_bc.offset,
                ap=[list(bo_bc.ap[0]), [0, 16], list(bo_bc.ap[1])],
            )
            nc.vector.tensor_add(out=xtb, in0=xt_all, in1=bo_bcast)

            # tail in two halves so transpose/normalize/store of half 0
            # overlap the z-projection of half 1
            z_all = tailp.tile([80, 2048], BF16, tag="z", bufs=1)
            zt_all = tailp.tile([128, 16, 128], BF16, tag="zt", bufs=1)
            r_all = yp.tile([128, 16], F32, tag="r", bufs=1)
            y_all = yp.tile([128, 16, C], F32, tag="y", bufs=1)
            y_view = y_d.rearrange("(j p) c -> p j c", p=128)
            for hh in range(2):
                for qc in (2 * hh, 2 * hh + 1):
                    ou = tailp.tile([128, 512], BF16, tag="ou")
                    if qc % 2:
                        nc.vector.tensor_copy(out=ou, in_=o_tiles[qc])
                    else:
                        nc.scalar.activation(
                            out=ou, in_=o_tiles[qc], func=AF.Copy
                        )
                    z_ps = sps.tile([128, 512], F32, tag="sps", name=f"z{qc}")
                    nc.tensor.matmul(
                        out=z_ps, lhsT=waug, rhs=ou, start=True, stop=True
                    )
                    if qc % 2:
                        nc.scalar.activation(
                            out=z_all[0 : C + 1, bass.ts(qc, 512)],
                            in_=z_ps[0 : C + 1, :],
                            func=AF.Copy,
                        )
                    else:
                        nc.vector.tensor_copy(
                            out=z_all[0 : C + 1, bass.ts(qc, 512)],
                            in_=z_ps[0 : C + 1, :],
                        )
                heng = nc.scalar if hh else nc.sync
                jsl = bass.ds(hh * 8, 8)
                heng.dma_start_transpose(
                    out=zt_all[:, jsl, 0:80],
                    in_=z_all[:, bass.ds(hh * 1024, 1024)],
                )
                nc.vector.reciprocal(
                    out=r_all[:, jsl], in_=zt_all[:, jsl, C]
                )
                for j in range(8 * hh, 8 * hh + 8):
                    nc.vector.scalar_tensor_tensor(
                        out=y_all[:, j, :], in0=zt_all[:, j, 0:C],
                        scalar=r_all[:, j : j + 1], in1=xtb[:, j, :],
                        op0=ALU.mult, op1=ALU.add,
                    )
                heng.dma_start(out=y_view[:, jsl, :], in_=y_all[:, jsl, :])
    return nc


_NC = None


def _get_nc():
    global _NC
    if _NC is None:
        _NC = build_nc()
    return _NC


def _prep_maps(x, Wq, bq, Wk, bk, Wv, bv, Wo, bo, gamma, beta):
    bf = ml_dtypes.bfloat16
    w_qk = np.concatenate([Wq.T, Wk.T], axis=1).astype(bf)  # [cin, 2c]
    wv_t = np.ascontiguousarray(Wv.T).astype(bf)
    w_aug = np.zeros((C + 1, C + 1), np.float32)
    w_aug[:C, :C] = Wo.T
    w_aug[C, C] = 1.0
    w_aug = w_aug.astype(bf)
    gbias = np.zeros((2 * C, 4), np.float32)
    gbias[:C, 0] = gamma
    gbias[:C, 1] = beta
    gbias[:, 2] = np.concatenate([bq, bk])
    gbias[:C, 3] = bv
    brows = np.zeros((1, 3 * C), np.float32)
    brows[0, : 2 * C] = np.concatenate([bq, bk])
    brows[0, 2 * C :] = bv
    bo_bc = np.tile(bo[None, :], (128, 1)).astype(np.float32)

    shared = dict(
        w_qk=w_qk, wv_t=wv_t, w_aug=w_aug, gbias=gbias, brows=brows,
        bo_bc=bo_bc,
    )
    in_maps = []
    for core in range(8):
        b, half = core // 2, core % 2
        xm = np.ascontiguousarray(x[b].reshape(C, N)).astype(np.float32)
        ones = np.ones((1, N), np.float32)
        xm1 = np.concatenate([xm, ones], axis=0)
        xqm = np.ascontiguousarray(xm1[:, half * NQ : (half + 1) * NQ])
        xtm = np.ascontiguousarray(xm.T[half * NQ : (half + 1) * NQ, :])
        in_maps.append(dict(shared, x=xm1, xq=xqm, xt=xtm))
    return in_maps


def run(inputs, trace=False):
    from concourse.bass_utils import run_bass_kernel_spmd

    inputs = {k: np.asarray(v) for k, v in inputs.items()}
    nc = _get_nc()
    in_maps = _prep_maps(**inputs)
    res = run_bass_kernel_spmd(
        nc, in_maps, core_ids=list(range(8)), trace=trace
    )
    out = np.empty((B, C, N), np.float32)
    for core in range(8):
        b, half = core // 2, core % 2
        out[b][:, half * NQ : (half + 1) * NQ] = res.results[core]["y"].T
    return out.reshape(B, C, H, W), res


def kernel(**inputs):
    out, _ = run(inputs, trace=False)
    return out



# revision 37
# speedup vs baseline: 1.4419x; 1.0093x over previous
"""Trainium2 Bass kernel for the AttentionBlock problem.

Fixed problem shape: x [4, 64, 64, 64] fp32, GroupNorm(32 groups) ->
1x1 conv Q/K/V -> softmax(Q^T K / 8) -> V @ attn^T -> 1x1 conv + residual.

Sharding: 8 cores, core = 2*batch + query_half. Each core holds its batch's
full x (for K/V) and computes outputs for its 2048-query half.

Layout strategy (per core):
  - x, K, Q, V live as [c=64 partitions, n free]; matmul operands in bf16
    with fp32 PSUM accumulation (attention is ~5% of the residual output,
    so final norm-rel-err stays ~1e-3).
  - GroupNorm stats via bn_stats + a DVE 32x32 stream-transpose pair-combine
    (PE-free); the affine fold goes INTO the projection weights
    (W*diag(s) stationaries) and a bias row (t/s)^T@(W*s)+b against a
    host-appended ones-row of x, so there is no normalization pass at all.
  - Scores are computed TRANSPOSED: S_T[k,q] = K_blk^T Q (contract c on
    partitions); the softmax denominator comes free from a ones-column
    appended to V^T during the PV matmul (no cross-partition reductions).
  - exp() runs on ScalarE directly out of PSUM in 1024-wide ops, no max
    subtraction (scores are O(+-10) here; exp stays well inside fp32 range).
  - PE stationary operands switch only twice per key block (K_blk for 4
    score matmuls, V^T_blk for 4 PV matmuls); PV for block kb-1 is emitted
    after the scores of kb (software pipeline).
  - V^T blocks and the output-projection transposes go through the DMA xbar
    transpose (bf16, 128B-aligned dst offsets), keeping them off the PE.
  - The PE clock on this part ramps 1.2->2.4 GHz only after sustained
    uninterrupted activity and re-throttles after idle gaps; a dummy
    same-weight warmup burst keeps the PE busy from kernel start through
    the projections so the attention loop can run warm (~125 us) instead
    of cold (~177 us) when the chip's power state cooperates.
  - Tail: augmented 65x65 Wo carries the denominators through the output
    projection; one xbar transpose puts q on partitions, one batched
    reciprocal + 16 fused multiply-adds apply 1/denom + residual + bias,
    one strided DMA writes y back.
"""

import numpy as np
import ml_dtypes

import concourse.bass as bass
import concourse.mybir as mybir
import concourse.tile as tile
from concourse.tile_rust import add_dep_helper
from concourse.vector_clock import ScopedClock

B, C, H, W = 4, 64, 64, 64
N = H * W            # 4096
NQ = N // 2          # queries per core
EPS = 1e-5
KB = 32              # key blocks of 128
WARMUP_REPS = 26     # initial PE warmup burst (more interleaved later)
F32 = mybir.dt.float32
BF16 = mybir.dt.bfloat16
AF = mybir.ActivationFunctionType
ALU = mybir.AluOpType


# ---------------------------------------------------------------------------
# This container's walrus codegen rejects >1 sync wait on one instruction
# ("Too many sync wait commands") — split extra waits onto preceding same-
# engine NOPs (engines execute in order, so semantics are preserved), and do
# the same for the TileContext tail drain.
def _install_drain_patch():
    if getattr(tile.TileContext, "_drain_patch_installed", False):
        return

    orig_commit = tile.TileContext._commit_instruction

    def _split_commit(self, inst, lazy_reg_writes=True):
        si = getattr(inst, "sync_info", None)
        if (
            si is not None
            and len(si.on_wait) > 1
            and inst.engine != mybir.EngineType.Unassigned
        ):
            waits = list(si.on_wait)
            inst.sync_info = mybir.SyncInfo(
                on_wait=waits[-1:], on_update=list(si.on_update)
            )
            for w in waits[:-1]:
                nop = mybir.InstNoOp(
                    name=self.nc.get_next_instruction_name(),
                    sync_info=mybir.SyncInfo(on_wait=[w], on_update=[]),
                    bass_nofuse=True,
                    engine=inst.engine,
                )
                orig_commit(self, nop, lazy_reg_writes=False)
        orig_commit(self, inst, lazy_reg_writes)

    def _patched(self, tick_clock, wait_clock):
        nc = self.nc
        drain_inst = nc.sync.drain()
        wait_clock.add_sem_waits(
            drain_inst.ins, ScopedClock({None: tick_clock.global_clock})
        )
        si = drain_inst.ins.sync_info
        if si is not None and len(si.on_wait) > 1:
            waits = list(si.on_wait)
            drain_inst.ins.sync_info = mybir.SyncInfo(
                on_wait=waits[:1], on_update=list(si.on_update)
            )
            for i in range(1, len(waits)):
                extra = nc.sync.drain()
                extra.ins.sync_info = mybir.SyncInfo(
                    on_wait=waits[i : i + 1], on_update=[]
                )
        nc.all_engine_barrier()
        assert self.sems is not None
        popped = nc._tile_sem_poison_stack.pop()
        assert popped is self._sem_poison
        nc.clear_and_free_semaphores(list(self.sems.allocated().values()))
        nc.all_engine_barrier()

    tile.TileContext._commit_instruction = _split_commit
    tile.TileContext._drain_and_barrier = _patched
    tile.TileContext._drain_patch_installed = True


def build_nc():
    _install_drain_patch()
    nc = bass.Bass()

    # per-core data
    # x / xq carry a host-appended ones row (row 64) for the bias-row trick
    x_d = nc.dram_tensor("x", [C + 1, N], F32, kind="ExternalInput")
    xq_d = nc.dram_tensor("xq", [C + 1, NQ], F32, kind="ExternalInput")
    xt_d = nc.dram_tensor("xt", [NQ, C], F32, kind="ExternalInput")
    # small zeros block for DMA zero-fills of padded partition ranges
    # (DMA has no 32-partition-alignment limit, so pads can start at 65
    # and never overlap the ones row the casts write)
    zp_d = nc.dram_tensor("zpad", [C, N], BF16, kind="ExternalInput")
    # replicated weights / constants
    wqk_d = nc.dram_tensor("w_qk", [C, 2 * C], BF16, kind="ExternalInput")
    wv_d = nc.dram_tensor("wv_t", [C, C], BF16, kind="ExternalInput")
    waug_d = nc.dram_tensor("w_aug", [C + 1, C + 1], BF16, kind="ExternalInput")
    # gbias columns: 0 gamma, 1 beta, 2 [bq;bk] stacked, 3 bv
    gb_d = nc.dram_tensor("gbias", [2 * C, 4], F32, kind="ExternalInput")
    brow_d = nc.dram_tensor("brows", [1, 3 * C], F32, kind="ExternalInput")
    bo_d = nc.dram_tensor("bo_bc", [128, C], F32, kind="ExternalInput")
    y_d = nc.dram_tensor("y", [NQ, C], F32, kind="ExternalOutput")

    with tile.TileContext(nc) as tc:
        with (
            tc.tile_pool(name="const", bufs=1) as const,
            tc.tile_pool(name="big", bufs=1) as big,
            tc.tile_pool(name="stats", bufs=2) as stats,
            tc.tile_pool(name="pt", bufs=4) as ptp,
            tc.tile_pool(name="tail", bufs=2) as tailp,
            tc.tile_pool(name="yp", bufs=3) as yp,
            tc.tile_pool(name="xtp", bufs=3) as xtp,
            tc.tile_pool(name="sps", bufs=2, space="PSUM") as sps,
            tc.tile_pool(name="ops", bufs=4, space="PSUM") as ops,
        ):
            # ---- load constants
            wqk = const.tile([C, 2 * C], BF16, tag="wqk")
            wv = const.tile([C, C], BF16, tag="wv")
            waug = const.tile([128, 128], BF16, tag="waug")
            gb = const.tile([2 * C, 4], F32, tag="gb")
            brow = const.tile([1, 3 * C], F32, tag="brow")
            bo_bc = const.tile([128, C], F32, tag="bo")
            gamma = gb[:C, 0:1]
            beta = gb[:C, 1:2]
            bqk_col = gb[:, 2:3]
            bv_col = gb[:C, 3:4]

            # ---- PE clock warmup: the PE clock ramps 1.2->2.4 GHz only
            # after sustained uninterrupted activity and then stays warm as
            # long as it never idles >~3us. Keep the PE busy with dummy
            # same-weight matmuls from kernel start until the projections are
            # ready (more reps are interleaved into the projection phase).
            eps_t = stats.tile([C, 1], F32, tag="eps")
            nc.vector.memset(eps_t, EPS)
            # first ACT instruction = tiny Exp: walrus attaches the one-time
            # ACT_TABLE_LOAD here, in the startup dead zone
            tblw = stats.tile([C, 1], F32, tag="tblw")
            nc.scalar.activation(out=tblw, in_=eps_t, func=AF.Exp)
            warm_sb = const.tile([128, 512], BF16, tag="warm")
            nc.vector.memset(warm_sb, 0.0)

            def warm_reps(n, base):
                for i in range(n):
                    wp = ops.tile([128, 512], F32, tag="o", name=f"w{base}_{i}")
                    nc.tensor.matmul(
                        out=wp, lhsT=warm_sb[:, 0:128], rhs=warm_sb,
                        start=True, stop=True,
                    )

            warm_reps(WARMUP_REPS, "a")

            # chain-independent prep, off the GroupNorm critical path
            tp1 = stats.tile([C, 32], F32, tag="tp1")
            nc.vector.memset(tp1, 0.0)
            g_inv = stats.tile([C, 1], F32, tag="ginv")
            nc.vector.memset(waug, 0.0)

            # ---- load x; bn_stats per 512-chunk as chunks arrive.
            # bf16 copies of x/xq for the projection matmuls come via
            # gpsimd cast-DMAs (GroupNorm scale/shift are folded into the
            # projection weights, so no separate normalization pass).
            x_sb = big.tile([C + 1, N], F32, tag="x")
            xq_sb = big.tile([C + 1, NQ], F32, tag="xq")
            # projection inputs padded to 128 partitions (rows 65:128 zero)
            # so every matmul enables the full 128x128 array: the HAM clock
            # gate integrates the ENABLED array fraction, and half-array
            # matmuls (contract=64 / 65 out rows) get throttled to 1.2 GHz
            # even when the PE has zero idle gaps.
            x_bf = big.tile([128, N], BF16, tag="xbf")
            xq_bf = big.tile([128, NQ], BF16, tag="xqbf")

            # issue ALL input-chunk DMAs first, all from the sync engine
            # (scalar's FIFO stalls behind the ACT table load; a dma_start
            # stuck there delays half the x chunks by ~5us)
            for j in range(8):
                sl = bass.ts(j, 512)
                nc.sync.dma_start(out=x_sb[:, sl], in_=x_d[:, sl])
            for j in range(4):
                sl = bass.ts(j, 512)
                nc.sync.dma_start(out=xq_sb[:, sl], in_=xq_d[:, sl])
            # zero-pads AFTER the chunks in queue order; [65:128] never
            # overlaps the ones row, so the casts don't wait on them
            nc.sync.dma_start(out=x_bf[C + 1 : 128, :], in_=zp_d[0:63, :])
            nc.sync.dma_start(
                out=xq_bf[C + 1 : 128, :], in_=zp_d[0:63, 0:NQ]
            )
            st = stats.tile([C, 8, 6], F32, tag="bnst")
            for j in range(8):
                sl = bass.ts(j, 512)
                nc.vector.bn_stats(out=st[:, j, :], in_=x_sb[0:C, sl])
                # bf16 cast for the projection inputs on ScalarE (idle
                # here; gpsimd casts are ~3x slower and stall concurrent
                # DVE ops via port sharing)
                nc.scalar.copy(out=x_bf[0 : C + 1, sl], in_=x_sb[:, sl])
            for j in range(4):
                sl = bass.ts(j, 512)
                nc.scalar.copy(out=xq_bf[0 : C + 1, sl], in_=xq_sb[:, sl])
            nc.sync.dma_start(out=wqk, in_=wqk_d[:, :])
            nc.sync.dma_start(out=wv, in_=wv_d[:, :])
            nc.sync.dma_start(out=waug[0 : C + 1, 0 : C + 1], in_=waug_d[:, :])
            nc.sync.dma_start(out=gb, in_=gb_d[:, :])
            nc.sync.dma_start(out=brow, in_=brow_d[:, :])
            nc.sync.dma_start(out=bo_bc, in_=bo_d[:, :])
            mv = stats.tile([C, 2], F32, tag="mv")
            nc.vector.bn_aggr(out=mv, in_=st)
            # me2 = [mean, var + mean^2] per channel
            me2 = stats.tile([C, 2], F32, tag="me2")
            nc.vector.tensor_copy(out=me2[:, 0:1], in_=mv[:, 0:1])
            m2 = stats.tile([C, 1], F32, tag="m2")
            nc.vector.tensor_mul(out=m2, in0=mv[:, 0:1], in1=mv[:, 0:1])
            nc.vector.tensor_add(out=me2[:, 1:2], in0=mv[:, 1:2], in1=m2)
            # group (channel-pair) sums of [mean, E[x^2]] without touching
            # the PE: 32x32 stream-transpose, add adjacent columns, replicate,
            # transpose back.
            nc.vector.tensor_copy(out=tp1[:, 0:2], in_=me2)
            tp2 = stats.tile([C, 32], F32, tag="tp2")
            nc.vector.transpose(out=tp2, in_=tp1)
            t2v = tp2.rearrange("p (g two) -> p g two", two=2)
            tp3 = stats.tile([C, 16], F32, tag="tp3")
            nc.vector.tensor_add(out=tp3, in0=t2v[:, :, 0], in1=t2v[:, :, 1])
            tp4 = stats.tile([C, 32], F32, tag="tp4")
            t4v = tp4.rearrange("p (g two) -> p g two", two=2)
            nc.vector.tensor_copy(out=t4v[:, :, 0], in_=tp3)
            nc.vector.tensor_copy(out=t4v[:, :, 1], in_=tp3)
            tp5 = stats.tile([C, 32], F32, tag="tp5")
            nc.vector.transpose(out=tp5, in_=tp4)
            # tp5[:, 0] = 2*mean_g, tp5[:, 1] = 2*E[x^2]_g per channel
            mean_g = stats.tile([C, 1], F32, tag="meang")
            nc.vector.tensor_scalar(
                out=mean_g, in0=tp5[:, 0:1], scalar1=0.5, scalar2=None,
                op0=ALU.mult,
            )
            varg = stats.tile([C, 1], F32, tag="varg")
            nc.vector.tensor_mul(out=varg, in0=mean_g, in1=mean_g)
            nc.vector.scalar_tensor_tensor(
                out=varg, in0=tp5[:, 1:2], scalar=0.5, in1=varg,
                op0=ALU.mult, op1=ALU.subtract,
            )
            # rstd = 1/sqrt(var+eps);  s = rstd*gamma;  t = beta - mean*s
            nc.scalar.activation(out=varg, in_=varg, func=AF.Sqrt, bias=eps_t)

            rstd = stats.tile([C, 1], F32, tag="rstd")
            nc.vector.reciprocal(out=rstd, in_=varg)
            s_col = stats.tile([C, 1], F32, tag="scol")
            nc.vector.tensor_mul(out=s_col, in0=rstd, in1=gamma)
            t_col = stats.tile([C, 1], F32, tag="tcol")
            nc.vector.tensor_mul(out=t_col, in0=mean_g, in1=s_col)
            nc.vector.tensor_tensor(out=t_col, in0=beta, in1=t_col, op=ALU.subtract)

            # fold the GroupNorm affine into the projections:
            #   W @ (x*s + t) = (W*diag(s)) @ x + (W @ t)
            # the W@t bias goes in as a 65th contraction row against the
            # ones-row in x_bf/xq_bf, so projection copies are bias-free
            wqk_s = const.tile([128, 2 * C], BF16, tag="wqks")
            nc.vector.memset(wqk_s[C:128, :], 0.0)
            nc.vector.tensor_scalar_mul(out=wqk_s[0:C, :], in0=wqk, scalar1=s_col)
            wv_s = const.tile([128, 128], BF16, tag="wvs")
            nc.vector.memset(wv_s, 0.0)
            nc.vector.tensor_scalar_mul(
                out=wv_s[0:C, 0:C], in0=wv, scalar1=s_col
            )
            # bias matmuls reuse the SCALED stationaries: W@t = (W*s)@(t/s)
            # t/s = t * std / gamma   (varg holds sqrt(var+eps) here)
            nc.vector.reciprocal(out=g_inv, in_=gamma)
            s_inv = stats.tile([C, 1], F32, tag="sinv")
            nc.vector.tensor_mul(out=s_inv, in0=varg, in1=g_inv)
            t_bf = stats.tile([C, 1], BF16, tag="tbf")
            nc.vector.tensor_mul(out=t_bf, in0=t_col, in1=s_inv)

            # ---- QK fused pass over xn (rows 0:64 = Q+bq, 64:128 = K+bk),
            #      V pass, all with one stationary each
            # K and Q padded to 128 partitions (zeros below row 64) so the
            # score matmuls contract over the full 128 array rows
            k_sb = big.tile([128, N], BF16, tag="k")
            q_sb = big.tile([128, NQ], BF16, tag="q")
            v_sb = big.tile([C, N], BF16, tag="v")
            nc.sync.dma_start(out=k_sb[C:128, :], in_=zp_d[:, :])
            nc.sync.dma_start(out=q_sb[C:128, :], in_=zp_d[:, 0:NQ])
            # bias rows: (t/s)^T @ (W*s) + b  -> row 64 of each stationary
            trow_ps = sps.tile([1, 2 * C], F32, tag="sps", name="trowqk")
            nc.tensor.matmul(
                out=trow_ps, lhsT=t_bf, rhs=wqk_s[0:C, :], start=True, stop=True
            )
            nc.vector.tensor_add(
                out=wqk_s[C : C + 1, :], in0=trow_ps, in1=brow[0:1, 0 : 2 * C]
            )
            tvow_ps = sps.tile([1, C], F32, tag="sps", name="trowv")
            nc.tensor.matmul(
                out=tvow_ps, lhsT=t_bf, rhs=wv_s[0:C, 0:C], start=True, stop=True
            )
            nc.vector.tensor_add(
                out=wv_s[C : C + 1, 0:C], in0=tvow_ps,
                in1=brow[0:1, 2 * C : 3 * C],
            )
            # one wqk_s-stationary run: 8 chunks of x (K rows) + 4 of xq (Q),
            # pure copies strictly alternating ACT/DVE so the PE never stalls
            for j in range(8):
                sl = bass.ts(j, 512)
                ps = ops.tile([128, 512], F32, tag="o", name=f"qk{j}")
                nc.tensor.matmul(
                    out=ps, lhsT=wqk_s, rhs=x_bf[:, sl], start=True, stop=True
                )
                if j % 2:
                    nc.scalar.activation(
                        out=k_sb[0:C, sl], in_=ps[64:128, :], func=AF.Copy
                    )
                else:
                    nc.vector.tensor_copy(out=k_sb[0:C, sl], in_=ps[64:128, :])
            for j in range(4):
                sl = bass.ts(j, 512)
                ps = ops.tile([128, 512], F32, tag="o", name=f"qq{j}")
                nc.tensor.matmul(
                    out=ps, lhsT=wqk_s, rhs=xq_bf[:, sl], start=True, stop=True
                )
                if j % 2:
                    nc.scalar.activation(
                        out=q_sb[0:C, sl], in_=ps[0:64, :], func=AF.Copy
                    )
                else:
                    nc.vector.tensor_copy(out=q_sb[0:C, sl], in_=ps[0:64, :])
            # then one wv_s-stationary run; V^T xbar transposes per pair so
            # the first key blocks are ready as the attention loop starts
            for j in range(8):
                sl = bass.ts(j, 512)
                ps = ops.tile([128, 512], F32, tag="o", name=f"v{j}")
                nc.tensor.matmul(
                    out=ps, lhsT=wv_s, rhs=x_bf[:, sl], start=True, stop=True
                )
                if j % 2:
                    nc.scalar.activation(
                        out=v_sb[:, sl], in_=ps[0:C, :], func=AF.Copy
                    )
                else:
                    nc.vector.tensor_copy(out=v_sb[:, sl], in_=ps[0:C, :])

            # ---- V^T blocks [128, 65] with ones column, via DMA xbar
            # per-block stride padded to 128 elements: the xbar transpose
            # needs 128B-aligned destination offsets. out[p, kb, c] =
            # V^T[kb*128 + p, c]; 4 chunked calls so early key blocks are
            # ready as soon as their v chunks are copied.
            vt = big.tile([128, KB, 128], BF16, tag="vt")
            # zero the whole vt tile (contiguous, 128 descriptors); the
            # transposes and the ones column overwrite their regions after.
            # Cols C+1:128 stay zero so PV matmuls enable all four 32-col
            # groups of the array (out rows 65:128 accumulate zeros)
            nc.gpsimd.memset(vt, 0.0)
            for t in range(4):
                nc.sync.dma_start_transpose(
                    out=vt[:, bass.ds(t * 8, 8), 0:C],
                    in_=v_sb[:, bass.ts(t, 1024)],
                )
            nc.vector.memset(vt[:, :, C : C + 1], 1.0)

            # ---- whole-xt load up front (tail residual input)
            xt_all = xtp.tile([128, 16, C], F32, tag="xt", bufs=1)
            nc.sync.dma_start(
                out=xt_all, in_=xt_d.rearrange("(j p) c -> p j c", p=128)
            )

            # ---- main attention loop
            o_tiles = [
                ops.tile([128, 512], F32, tag="o", name=f"o{qc}")
                for qc in range(4)
            ]
            # software-pipelined: PV for block kb-1 is emitted after the score
            # matmuls for block kb, so the PE does 4 same-stationary matmuls
            # per weight switch and exp(kb-1) has a full block to finish.
            def emit_pv(kb, p2, after):
                for qc in range(4):
                    mm = nc.tensor.matmul(
                        out=o_tiles[qc], lhsT=vt[:, kb, 0:128],
                        rhs=p2[qc // 2][:, (qc % 2) * 512 : (qc % 2 + 1) * 512],
                        start=(kb == 0), stop=(kb == KB - 1),
                        skip_group_check=True,
                    )
                    if qc == 0 and after is not None:
                        # keep the PE stream in same-stationary runs of 4:
                        # PV(kb-1) only after the last score matmul of kb
                        add_dep_helper(
                            mm.ins, after.ins, sync=False,
                            reason="group PE same-stationary runs",
                        )

            prev = None
            for kb in range(KB):
                kblk = k_sb[:, bass.ts(kb, 128)]
                s2 = []
                last_s = None
                for h in range(2):
                    sp = sps.tile([128, 1024], F32, tag="sps", name=f"s{kb}_{h}")
                    nc.tensor.matmul(
                        out=sp[:, 0:512], lhsT=kblk,
                        rhs=q_sb[:, bass.ds(h * 1024, 512)],
                        start=True, stop=True,
                    )
                    last_s = nc.tensor.matmul(
                        out=sp[:, 512:1024], lhsT=kblk,
                        rhs=q_sb[:, bass.ds(h * 1024 + 512, 512)],
                        start=True, stop=True,
                    )
                    s2.append(sp)
                p2 = []
                for h in range(2):
                    p = ptp.tile([128, 1024], BF16, tag="p", name=f"p{kb}_{h}")
                    nc.scalar.activation(out=p, in_=s2[h], func=AF.Exp, scale=0.125)
                    p2.append(p)
                if kb in (1, 2):
                    # pad the PE over exp(0)'s table load + latency so the
                    # pipeline fill doesn't leave a clock-dropping idle gap
                    warm_reps(3, f"fill{kb}")
                if prev is not None:
                    emit_pv(kb - 1, prev, last_s)
                prev = p2
            emit_pv(KB - 1, prev, None)

            # residual+bias prep on DVE, emitted here so it runs during the
            # loop (DVE is idle) instead of delaying the GroupNorm chain
            xtb = xtp.tile([128, 16, C], F32, tag="xtb", bufs=1)
            bo_bcast = bass.AP(
                tensor=bo_bc.tensor, offset=bo_bc.offset,
                ap=[list(bo_bc.ap[0]), [0, 16], list(bo_bc.ap[1])],
            )
            nc.vector.tensor_add(out=xtb, in0=xt_all, in1=bo_bcast)

            # ---- tail: project through augmented Wo, DMA-transpose,
            #      normalize by denominator, add residual + bo, store
            # xt was loaded up front (xt_all); add bo once, broadcast over j

            # tail in two halves so transpose/normalize/store of half 0
            # overlap the z-projection of half 1
            z_all = tailp.tile([80, 2048], BF16, tag="z", bufs=1)
            zt_all = tailp.tile([128, 16, 128], BF16, tag="zt", bufs=1)
            r_all = yp.tile([128, 16], F32, tag="r", bufs=1)
            y_all = yp.tile([128, 16, C], F32, tag="y", bufs=1)
            y_view = y_d.rearrange("(j p) c -> p j c", p=128)
            for hh in range(2):
                for qc in (2 * hh, 2 * hh + 1):
                    ou = tailp.tile([128, 512], BF16, tag="ou")
                    if qc % 2:
                        nc.vector.tensor_copy(out=ou, in_=o_tiles[qc])
                    else:
                        nc.scalar.activation(
                            out=ou, in_=o_tiles[qc], func=AF.Copy
                        )
                    z_ps = sps.tile([128, 512], F32, tag="sps", name=f"z{qc}")
                    nc.tensor.matmul(
                        out=z_ps, lhsT=waug, rhs=ou, start=True, stop=True
                    )
                    if qc % 2:
                        nc.scalar.activation(
                            out=z_all[0 : C + 1, bass.ts(qc, 512)],
                            in_=z_ps[0 : C + 1, :],
                            func=AF.Copy,
                        )
                    else:
                        nc.vector.tensor_copy(
                            out=z_all[0 : C + 1, bass.ts(qc, 512)],
                            in_=z_ps[0 : C + 1, :],
                        )
                heng = nc.scalar if hh else nc.sync
                jsl = bass.ds(hh * 8, 8)
                heng.dma_start_transpose(
                    out=zt_all[:, jsl, 0:80],
                    in_=z_all[:, bass.ds(hh * 1024, 1024)],
                )
                nc.vector.reciprocal(
                    out=r_all[:, jsl], in_=zt_all[:, jsl, C]
                )
                for j in range(8 * hh, 8 * hh + 8):
                    nc.vector.scalar_tensor_tensor(
                        out=y_all[:, j, :], in0=zt_all[:, j, 0:C],
                        scalar=r_all[:, j : j + 1], in1=xtb[:, j, :],
                        op0=ALU.mult, op1=ALU.add,
                    )
                heng.dma_start(out=y_view[:, jsl, :], in_=y_all[:, jsl, :])
    return nc


_NC = None


def _get_nc():
    global _NC
    if _NC is None:
        _NC = build_nc()
    return _NC


def _prep_maps(x, Wq, bq, Wk, bk, Wv, bv, Wo, bo, gamma, beta):
    bf = ml_dtypes.bfloat16
    w_qk = np.concatenate([Wq.T, Wk.T], axis=1).astype(bf)  # [cin, 2c]
    wv_t = np.ascontiguousarray(Wv.T).astype(bf)
    w_aug = np.zeros((C + 1, C + 1), np.float32)
    w_aug[:C, :C] = Wo.T
    w_aug[C, C] = 1.0
    w_aug = w_aug.astype(bf)
    gbias = np.zeros((2 * C, 4), np.float32)
    gbias[:C, 0] = gamma
    gbias[:C, 1] = beta
    gbias[:, 2] = np.concatenate([bq, bk])
    gbias[:C, 3] = bv
    brows = np.zeros((1, 3 * C), np.float32)
    brows[0, : 2 * C] = np.concatenate([bq, bk])
    brows[0, 2 * C :] = bv
    bo_bc = np.tile(bo[None, :], (128, 1)).astype(np.float32)

    shared = dict(
        w_qk=w_qk, wv_t=wv_t, w_aug=w_aug, gbias=gbias, brows=brows,
        bo_bc=bo_bc, zpad=np.zeros((C, N), bf),
    )
    in_maps = []
    for core in range(8):
        b, half = core // 2, core % 2
        xm = np.ascontiguousarray(x[b].reshape(C, N)).astype(np.float32)
        ones = np.ones((1, N), np.float32)
        xm1 = np.concatenate([xm, ones], axis=0)
        xqm = np.ascontiguousarray(xm1[:, half * NQ : (half + 1) * NQ])
        xtm = np.ascontiguousarray(xm.T[half * NQ : (half + 1) * NQ, :])
        in_maps.append(dict(shared, x=xm1, xq=xqm, xt=xtm))
    return in_maps


def run(inputs, trace=False):
    from concourse.bass_utils import run_bass_kernel_spmd

    inputs = {k: np.asarray(v) for k, v in inputs.items()}
    nc = _get_nc()
    in_maps = _prep_maps(**inputs)
    res = run_bass_kernel_spmd(
        nc, in_maps, core_ids=list(range(8)), trace=trace
    )
    out = np.empty((B, C, N), np.float32)
    for core in range(8):
        b, half = core // 2, core % 2
        out[b][:, half * NQ : (half + 1) * NQ] = res.results[core]["y"].T
    return out.reshape(B, C, H, W), res


def kernel(**inputs):
    out, _ = run(inputs, trace=False)
    return out



# revision 38
# speedup vs baseline: 1.4537x; 1.0082x over previous
"""Trainium2 Bass kernel for the AttentionBlock problem.

Fixed problem shape: x [4, 64, 64, 64] fp32, GroupNorm(32 groups) ->
1x1 conv Q/K/V -> softmax(Q^T K / 8) -> V @ attn^T -> 1x1 conv + residual.

Sharding: 8 cores, core = 2*batch + query_half. Each core holds its batch's
full x (for K/V) and computes outputs for its 2048-query half.

Layout strategy (per core):
  - x, K, Q, V live as [c=64 partitions, n free]; matmul operands in bf16
    with fp32 PSUM accumulation (attention is ~5% of the residual output,
    so final norm-rel-err stays ~1e-3).
  - GroupNorm stats via bn_stats + a DVE 32x32 stream-transpose pair-combine
    (PE-free); the affine fold goes INTO the projection weights
    (W*diag(s) stationaries) and a bias row (t/s)^T@(W*s)+b against a
    host-appended ones-row of x, so there is no normalization pass at all.
  - Scores are computed TRANSPOSED: S_T[k,q] = K_blk^T Q (contract c on
    partitions); the softmax denominator comes free from a ones-column
    appended to V^T during the PV matmul (no cross-partition reductions).
  - exp() runs on ScalarE directly out of PSUM in 1024-wide ops, no max
    subtraction (scores are O(+-10) here; exp stays well inside fp32 range).
  - PE stationary operands switch only twice per key block (K_blk for 4
    score matmuls, V^T_blk for 4 PV matmuls); PV for block kb-1 is emitted
    after the scores of kb (software pipeline).
  - V^T blocks and the output-projection transposes go through the DMA xbar
    transpose (bf16, 128B-aligned dst offsets), keeping them off the PE.
  - The PE clock on this part ramps 1.2->2.4 GHz only after sustained
    uninterrupted activity and re-throttles after idle gaps; a dummy
    same-weight warmup burst keeps the PE busy from kernel start through
    the projections so the attention loop can run warm (~125 us) instead
    of cold (~177 us) when the chip's power state cooperates.
  - Tail: augmented 65x65 Wo carries the denominators through the output
    projection; one xbar transpose puts q on partitions, one batched
    reciprocal + 16 fused multiply-adds apply 1/denom + residual + bias,
    one strided DMA writes y back.
"""

import numpy as np
import ml_dtypes

import concourse.bass as bass
import concourse.mybir as mybir
import concourse.tile as tile
from concourse.tile_rust import add_dep_helper
from concourse.vector_clock import ScopedClock

B, C, H, W = 4, 64, 64, 64
N = H * W            # 4096
NQ = N // 2          # queries per core
EPS = 1e-5
KB = 32              # key blocks of 128
WARMUP_REPS = 26     # initial PE warmup burst (more interleaved later)
F32 = mybir.dt.float32
BF16 = mybir.dt.bfloat16
AF = mybir.ActivationFunctionType
ALU = mybir.AluOpType


# ---------------------------------------------------------------------------
# This container's walrus codegen rejects >1 sync wait on one instruction
# ("Too many sync wait commands") — split extra waits onto preceding same-
# engine NOPs (engines execute in order, so semantics are preserved), and do
# the same for the TileContext tail drain.
def _install_drain_patch():
    if getattr(tile.TileContext, "_drain_patch_installed", False):
        return

    orig_commit = tile.TileContext._commit_instruction

    def _split_commit(self, inst, lazy_reg_writes=True):
        si = getattr(inst, "sync_info", None)
        if (
            si is not None
            and len(si.on_wait) > 1
            and inst.engine != mybir.EngineType.Unassigned
        ):
            waits = list(si.on_wait)
            inst.sync_info = mybir.SyncInfo(
                on_wait=waits[-1:], on_update=list(si.on_update)
            )
            for w in waits[:-1]:
                nop = mybir.InstNoOp(
                    name=self.nc.get_next_instruction_name(),
                    sync_info=mybir.SyncInfo(on_wait=[w], on_update=[]),
                    bass_nofuse=True,
                    engine=inst.engine,
                )
                orig_commit(self, nop, lazy_reg_writes=False)
        orig_commit(self, inst, lazy_reg_writes)

    def _patched(self, tick_clock, wait_clock):
        nc = self.nc
        drain_inst = nc.sync.drain()
        wait_clock.add_sem_waits(
            drain_inst.ins, ScopedClock({None: tick_clock.global_clock})
        )
        si = drain_inst.ins.sync_info
        if si is not None and len(si.on_wait) > 1:
            waits = list(si.on_wait)
            drain_inst.ins.sync_info = mybir.SyncInfo(
                on_wait=waits[:1], on_update=list(si.on_update)
            )
            for i in range(1, len(waits)):
                extra = nc.sync.drain()
                extra.ins.sync_info = mybir.SyncInfo(
                    on_wait=waits[i : i + 1], on_update=[]
                )
        nc.all_engine_barrier()
        assert self.sems is not None
        popped = nc._tile_sem_poison_stack.pop()
        assert popped is self._sem_poison
        nc.clear_and_free_semaphores(list(self.sems.allocated().values()))
        nc.all_engine_barrier()

    tile.TileContext._commit_instruction = _split_commit
    tile.TileContext._drain_and_barrier = _patched
    tile.TileContext._drain_patch_installed = True


def build_nc():
    _install_drain_patch()
    nc = bass.Bass()

    # per-core data
    # x / xq carry a host-appended ones row (row 64) for the bias-row trick
    x_d = nc.dram_tensor("x", [C + 1, N], F32, kind="ExternalInput")
    xq_d = nc.dram_tensor("xq", [C + 1, NQ], F32, kind="ExternalInput")
    xt_d = nc.dram_tensor("xt", [NQ, C], F32, kind="ExternalInput")
    # small zeros block for DMA zero-fills of padded partition ranges
    # (DMA has no 32-partition-alignment limit, so pads can start at 65
    # and never overlap the ones row the casts write)
    zp_d = nc.dram_tensor("zpad", [C, N], BF16, kind="ExternalInput")
    # replicated weights / constants
    wqk_d = nc.dram_tensor("w_qk", [C, 2 * C], BF16, kind="ExternalInput")
    wv_d = nc.dram_tensor("wv_t", [C, C], BF16, kind="ExternalInput")
    waug_d = nc.dram_tensor("w_aug", [C + 1, C + 1], BF16, kind="ExternalInput")
    # gbias columns: 0 gamma, 1 beta, 2 [bq;bk] stacked, 3 bv
    gb_d = nc.dram_tensor("gbias", [2 * C, 4], F32, kind="ExternalInput")
    brow_d = nc.dram_tensor("brows", [1, 3 * C], F32, kind="ExternalInput")
    bo_d = nc.dram_tensor("bo_bc", [128, C], F32, kind="ExternalInput")
    y_d = nc.dram_tensor("y", [NQ, C], F32, kind="ExternalOutput")

    with tile.TileContext(nc) as tc:
        with (
            tc.tile_pool(name="const", bufs=1) as const,
            tc.tile_pool(name="big", bufs=1) as big,
            tc.tile_pool(name="stats", bufs=2) as stats,
            tc.tile_pool(name="pt", bufs=4) as ptp,
            tc.tile_pool(name="tail", bufs=2) as tailp,
            tc.tile_pool(name="yp", bufs=3) as yp,
            tc.tile_pool(name="xtp", bufs=3) as xtp,
            tc.tile_pool(name="sps", bufs=2, space="PSUM") as sps,
            tc.tile_pool(name="ops", bufs=4, space="PSUM") as ops,
        ):
            # ---- load constants
            wqk = const.tile([C, 2 * C], BF16, tag="wqk")
            wv = const.tile([C, C], BF16, tag="wv")
            waug = const.tile([128, 128], BF16, tag="waug")
            gb = const.tile([2 * C, 4], F32, tag="gb")
            brow = const.tile([1, 3 * C], F32, tag="brow")
            bo_bc = const.tile([128, C], F32, tag="bo")
            gamma = gb[:C, 0:1]
            beta = gb[:C, 1:2]
            bqk_col = gb[:, 2:3]
            bv_col = gb[:C, 3:4]

            # ---- PE clock warmup: the PE clock ramps 1.2->2.4 GHz only
            # after sustained uninterrupted activity and then stays warm as
            # long as it never idles >~3us. Keep the PE busy with dummy
            # same-weight matmuls from kernel start until the projections are
            # ready (more reps are interleaved into the projection phase).
            x_sb = big.tile([C + 1, N], F32, tag="x")
            xq_sb = big.tile([C + 1, NQ], F32, tag="xq")
            # x chunk DMAs first on both scalar and sync FIFO heads (before
            # the ACT table preload) so transfers start at queue-arming time
            for j in range(8):
                sl = bass.ts(j, 512)
                eng = nc.scalar if j < 2 else nc.sync
                eng.dma_start(out=x_sb[:, sl], in_=x_d[:, sl])
            for j in range(4):
                sl = bass.ts(j, 512)
                nc.sync.dma_start(out=xq_sb[:, sl], in_=xq_d[:, sl])
            # replicated weights next in queue order (small, needed by ~15us)
            nc.sync.dma_start(out=wqk, in_=wqk_d[:, :])
            nc.sync.dma_start(out=wv, in_=wv_d[:, :])
            nc.sync.dma_start(out=gb, in_=gb_d[:, :])
            nc.sync.dma_start(out=brow, in_=brow_d[:, :])
            nc.sync.dma_start(out=bo_bc, in_=bo_d[:, :])
            eps_t = stats.tile([C, 1], F32, tag="eps")
            nc.vector.memset(eps_t, EPS)
            # first ACT instruction = tiny Exp: walrus attaches the one-time
            # ACT_TABLE_LOAD here, in the startup dead zone
            tblw = stats.tile([C, 1], F32, tag="tblw")
            nc.scalar.activation(out=tblw, in_=eps_t, func=AF.Exp)
            warm_sb = const.tile([128, 512], BF16, tag="warm")
            nc.vector.memset(warm_sb, 0.0)

            def warm_reps(n, base):
                for i in range(n):
                    wp = ops.tile([128, 512], F32, tag="o", name=f"w{base}_{i}")
                    nc.tensor.matmul(
                        out=wp, lhsT=warm_sb[:, 0:128], rhs=warm_sb,
                        start=True, stop=True,
                    )

            warm_reps(WARMUP_REPS, "a")

            # chain-independent prep, off the GroupNorm critical path
            tp1 = stats.tile([C, 32], F32, tag="tp1")
            nc.vector.memset(tp1, 0.0)
            g_inv = stats.tile([C, 1], F32, tag="ginv")
            nc.vector.memset(waug, 0.0)

            # ---- load x; bn_stats per 512-chunk as chunks arrive.
            # bf16 copies of x/xq for the projection matmuls come via
            # gpsimd cast-DMAs (GroupNorm scale/shift are folded into the
            # projection weights, so no separate normalization pass).
            # projection inputs padded to 128 partitions (rows 65:128 zero)
            # so every matmul enables the full 128x128 array: the HAM clock
            # gate integrates the ENABLED array fraction, and half-array
            # matmuls (contract=64 / 65 out rows) get throttled to 1.2 GHz
            # even when the PE has zero idle gaps.
            x_bf = big.tile([128, N], BF16, tag="xbf")
            xq_bf = big.tile([128, NQ], BF16, tag="xqbf")

            # zero-pads after chunks+weights in queue order; [65:128] never
            # overlaps the ones row, so the casts don't wait on them
            nc.sync.dma_start(out=x_bf[C + 1 : 128, :], in_=zp_d[0:63, :])
            nc.sync.dma_start(
                out=xq_bf[C + 1 : 128, :], in_=zp_d[0:63, 0:NQ]
            )
            st = stats.tile([C, 8, 6], F32, tag="bnst")
            for j in range(8):
                sl = bass.ts(j, 512)
                nc.vector.bn_stats(out=st[:, j, :], in_=x_sb[0:C, sl])
                # bf16 cast for the projection inputs on ScalarE (idle
                # here; gpsimd casts are ~3x slower and stall concurrent
                # DVE ops via port sharing)
                nc.scalar.copy(out=x_bf[0 : C + 1, sl], in_=x_sb[:, sl])
            for j in range(4):
                sl = bass.ts(j, 512)
                nc.scalar.copy(out=xq_bf[0 : C + 1, sl], in_=xq_sb[:, sl])
            nc.sync.dma_start(out=waug[0 : C + 1, 0 : C + 1], in_=waug_d[:, :])
            mv = stats.tile([C, 2], F32, tag="mv")
            nc.vector.bn_aggr(out=mv, in_=st)
            # me2 = [mean, var + mean^2] per channel
            me2 = stats.tile([C, 2], F32, tag="me2")
            nc.vector.tensor_copy(out=me2[:, 0:1], in_=mv[:, 0:1])
            m2 = stats.tile([C, 1], F32, tag="m2")
            nc.vector.tensor_mul(out=m2, in0=mv[:, 0:1], in1=mv[:, 0:1])
            nc.vector.tensor_add(out=me2[:, 1:2], in0=mv[:, 1:2], in1=m2)
            # group (channel-pair) sums of [mean, E[x^2]] without touching
            # the PE: 32x32 stream-transpose, add adjacent columns, replicate,
            # transpose back.
            nc.vector.tensor_copy(out=tp1[:, 0:2], in_=me2)
            tp2 = stats.tile([C, 32], F32, tag="tp2")
            nc.vector.transpose(out=tp2, in_=tp1)
            t2v = tp2.rearrange("p (g two) -> p g two", two=2)
            tp3 = stats.tile([C, 16], F32, tag="tp3")
            nc.vector.tensor_add(out=tp3, in0=t2v[:, :, 0], in1=t2v[:, :, 1])
            tp4 = stats.tile([C, 32], F32, tag="tp4")
            t4v = tp4.rearrange("p (g two) -> p g two", two=2)
            nc.vector.tensor_copy(out=t4v[:, :, 0], in_=tp3)
            nc.vector.tensor_copy(out=t4v[:, :, 1], in_=tp3)
            tp5 = stats.tile([C, 32], F32, tag="tp5")
            nc.vector.transpose(out=tp5, in_=tp4)
            # tp5[:, 0] = 2*mean_g, tp5[:, 1] = 2*E[x^2]_g per channel
            mean_g = stats.tile([C, 1], F32, tag="meang")
            nc.vector.tensor_scalar(
                out=mean_g, in0=tp5[:, 0:1], scalar1=0.5, scalar2=None,
                op0=ALU.mult,
            )
            varg = stats.tile([C, 1], F32, tag="varg")
            nc.vector.tensor_mul(out=varg, in0=mean_g, in1=mean_g)
            nc.vector.scalar_tensor_tensor(
                out=varg, in0=tp5[:, 1:2], scalar=0.5, in1=varg,
                op0=ALU.mult, op1=ALU.subtract,
            )
            # rstd = 1/sqrt(var+eps);  s = rstd*gamma;  t = beta - mean*s
            nc.scalar.activation(out=varg, in_=varg, func=AF.Sqrt, bias=eps_t)

            rstd = stats.tile([C, 1], F32, tag="rstd")
            nc.vector.reciprocal(out=rstd, in_=varg)
            s_col = stats.tile([C, 1], F32, tag="scol")
            nc.vector.tensor_mul(out=s_col, in0=rstd, in1=gamma)
            t_col = stats.tile([C, 1], F32, tag="tcol")
            nc.vector.tensor_mul(out=t_col, in0=mean_g, in1=s_col)
            nc.vector.tensor_tensor(out=t_col, in0=beta, in1=t_col, op=ALU.subtract)

            # fold the GroupNorm affine into the projections:
            #   W @ (x*s + t) = (W*diag(s)) @ x + (W @ t)
            # the W@t bias goes in as a 65th contraction row against the
            # ones-row in x_bf/xq_bf, so projection copies are bias-free
            wqk_s = const.tile([128, 2 * C], BF16, tag="wqks")
            nc.vector.memset(wqk_s[C:128, :], 0.0)
            nc.vector.tensor_scalar_mul(out=wqk_s[0:C, :], in0=wqk, scalar1=s_col)
            wv_s = const.tile([128, 128], BF16, tag="wvs")
            nc.vector.memset(wv_s, 0.0)
            nc.vector.tensor_scalar_mul(
                out=wv_s[0:C, 0:C], in0=wv, scalar1=s_col
            )
            # bias matmuls reuse the SCALED stationaries: W@t = (W*s)@(t/s)
            # t/s = t * std / gamma   (varg holds sqrt(var+eps) here)
            nc.vector.reciprocal(out=g_inv, in_=gamma)
            s_inv = stats.tile([C, 1], F32, tag="sinv")
            nc.vector.tensor_mul(out=s_inv, in0=varg, in1=g_inv)
            t_bf = stats.tile([C, 1], BF16, tag="tbf")
            nc.vector.tensor_mul(out=t_bf, in0=t_col, in1=s_inv)

            # ---- QK fused pass over xn (rows 0:64 = Q+bq, 64:128 = K+bk),
            #      V pass, all with one stationary each
            # K and Q padded to 128 partitions (zeros below row 64) so the
            # score matmuls contract over the full 128 array rows
            k_sb = big.tile([128, N], BF16, tag="k")
            q_sb = big.tile([128, NQ], BF16, tag="q")
            v_sb = big.tile([C, N], BF16, tag="v")
            nc.sync.dma_start(out=k_sb[C:128, :], in_=zp_d[:, :])
            nc.sync.dma_start(out=q_sb[C:128, :], in_=zp_d[:, 0:NQ])
            # bias rows: (t/s)^T @ (W*s) + b  -> row 64 of each stationary
            trow_ps = sps.tile([1, 2 * C], F32, tag="sps", name="trowqk")
            nc.tensor.matmul(
                out=trow_ps, lhsT=t_bf, rhs=wqk_s[0:C, :], start=True, stop=True
            )
            nc.vector.tensor_add(
                out=wqk_s[C : C + 1, :], in0=trow_ps, in1=brow[0:1, 0 : 2 * C]
            )
            tvow_ps = sps.tile([1, C], F32, tag="sps", name="trowv")
            nc.tensor.matmul(
                out=tvow_ps, lhsT=t_bf, rhs=wv_s[0:C, 0:C], start=True, stop=True
            )
            nc.vector.tensor_add(
                out=wv_s[C : C + 1, 0:C], in0=tvow_ps,
                in1=brow[0:1, 2 * C : 3 * C],
            )
            # one wqk_s-stationary run: 8 chunks of x (K rows) + 4 of xq (Q),
            # pure copies strictly alternating ACT/DVE so the PE never stalls
            for j in range(8):
                sl = bass.ts(j, 512)
                ps = ops.tile([128, 512], F32, tag="o", name=f"qk{j}")
                nc.tensor.matmul(
                    out=ps, lhsT=wqk_s, rhs=x_bf[:, sl], start=True, stop=True
                )
                if j % 2:
                    nc.scalar.activation(
                        out=k_sb[0:C, sl], in_=ps[64:128, :], func=AF.Copy
                    )
                else:
                    nc.vector.tensor_copy(out=k_sb[0:C, sl], in_=ps[64:128, :])
            for j in range(4):
                sl = bass.ts(j, 512)
                ps = ops.tile([128, 512], F32, tag="o", name=f"qq{j}")
                nc.tensor.matmul(
                    out=ps, lhsT=wqk_s, rhs=xq_bf[:, sl], start=True, stop=True
                )
                if j % 2:
                    nc.scalar.activation(
                        out=q_sb[0:C, sl], in_=ps[0:64, :], func=AF.Copy
                    )
                else:
                    nc.vector.tensor_copy(out=q_sb[0:C, sl], in_=ps[0:64, :])
            # then one wv_s-stationary run; V^T xbar transposes per pair so
            # the first key blocks are ready as the attention loop starts
            for j in range(8):
                sl = bass.ts(j, 512)
                ps = ops.tile([128, 512], F32, tag="o", name=f"v{j}")
                nc.tensor.matmul(
                    out=ps, lhsT=wv_s, rhs=x_bf[:, sl], start=True, stop=True
                )
                if j % 2:
                    nc.scalar.activation(
                        out=v_sb[:, sl], in_=ps[0:C, :], func=AF.Copy
                    )
                else:
                    nc.vector.tensor_copy(out=v_sb[:, sl], in_=ps[0:C, :])

            # ---- V^T blocks [128, 65] with ones column, via DMA xbar
            # per-block stride padded to 128 elements: the xbar transpose
            # needs 128B-aligned destination offsets. out[p, kb, c] =
            # V^T[kb*128 + p, c]; 4 chunked calls so early key blocks are
            # ready as soon as their v chunks are copied.
            vt = big.tile([128, KB, 128], BF16, tag="vt")
            # zero the whole vt tile (contiguous, 128 descriptors); the
            # transposes and the ones column overwrite their regions after.
            # Cols C+1:128 stay zero so PV matmuls enable all four 32-col
            # groups of the array (out rows 65:128 accumulate zeros)
            nc.gpsimd.memset(vt, 0.0)
            for t in range(4):
                nc.sync.dma_start_transpose(
                    out=vt[:, bass.ds(t * 8, 8), 0:C],
                    in_=v_sb[:, bass.ts(t, 1024)],
                )
            nc.vector.memset(vt[:, :, C : C + 1], 1.0)

            # ---- whole-xt load up front (tail residual input)
            xt_all = xtp.tile([128, 16, C], F32, tag="xt", bufs=1)
            nc.sync.dma_start(
                out=xt_all, in_=xt_d.rearrange("(j p) c -> p j c", p=128)
            )

            # ---- main attention loop
            o_tiles = [
                ops.tile([128, 512], F32, tag="o", name=f"o{qc}")
                for qc in range(4)
            ]
            # software-pipelined: PV for block kb-1 is emitted after the score
            # matmuls for block kb, so the PE does 4 same-stationary matmuls
            # per weight switch and exp(kb-1) has a full block to finish.
            def emit_pv(kb, p2, after):
                for qc in range(4):
                    mm = nc.tensor.matmul(
                        out=o_tiles[qc], lhsT=vt[:, kb, 0:128],
                        rhs=p2[qc // 2][:, (qc % 2) * 512 : (qc % 2 + 1) * 512],
                        start=(kb == 0), stop=(kb == KB - 1),
                        skip_group_check=True,
                    )
                    if qc == 0 and after is not None:
                        # keep the PE stream in same-stationary runs of 4:
                        # PV(kb-1) only after the last score matmul of kb
                        add_dep_helper(
                            mm.ins, after.ins, sync=False,
                            reason="group PE same-stationary runs",
                        )

            prev = None
            for kb in range(KB):
                kblk = k_sb[:, bass.ts(kb, 128)]
                s2 = []
                last_s = None
                for h in range(2):
                    sp = sps.tile([128, 1024], F32, tag="sps", name=f"s{kb}_{h}")
                    nc.tensor.matmul(
                        out=sp[:, 0:512], lhsT=kblk,
                        rhs=q_sb[:, bass.ds(h * 1024, 512)],
                        start=True, stop=True,
                    )
                    last_s = nc.tensor.matmul(
                        out=sp[:, 512:1024], lhsT=kblk,
                        rhs=q_sb[:, bass.ds(h * 1024 + 512, 512)],
                        start=True, stop=True,
                    )
                    s2.append(sp)
                p2 = []
                for h in range(2):
                    p = ptp.tile([128, 1024], BF16, tag="p", name=f"p{kb}_{h}")
                    nc.scalar.activation(out=p, in_=s2[h], func=AF.Exp, scale=0.125)
                    p2.append(p)
                if kb in (1, 2):
                    # pad the PE over exp(0)'s table load + latency so the
                    # pipeline fill doesn't leave a clock-dropping idle gap
                    warm_reps(3, f"fill{kb}")
                if prev is not None:
                    emit_pv(kb - 1, prev, last_s)
                prev = p2
            emit_pv(KB - 1, prev, None)

            # residual+bias prep on DVE, emitted here so it runs during the
            # loop (DVE is idle) instead of delaying the GroupNorm chain
            xtb = xtp.tile([128, 16, C], F32, tag="xtb", bufs=1)
            bo_bcast = bass.AP(
                tensor=bo_bc.tensor, offset=bo_bc.offset,
                ap=[list(bo_bc.ap[0]), [0, 16], list(bo_bc.ap[1])],
            )
            nc.vector.tensor_add(out=xtb, in0=xt_all, in1=bo_bcast)

            # ---- tail: project through augmented Wo, DMA-transpose,
            #      normalize by denominator, add residual + bo, store
            # xt was loaded up front (xt_all); add bo once, broadcast over j

            # tail in two halves so transpose/normalize/store of half 0
            # overlap the z-projection of half 1
            z_all = tailp.tile([80, 2048], BF16, tag="z", bufs=1)
            zt_all = tailp.tile([128, 16, 128], BF16, tag="zt", bufs=1)
            r_all = yp.tile([128, 16], F32, tag="r", bufs=1)
            y_all = yp.tile([128, 16, C], F32, tag="y", bufs=1)
            y_view = y_d.rearrange("(j p) c -> p j c", p=128)
            for hh in range(2):
                for qc in (2 * hh, 2 * hh + 1):
                    ou = tailp.tile([128, 512], BF16, tag="ou")
                    if qc % 2:
                        nc.vector.tensor_copy(out=ou, in_=o_tiles[qc])
                    else:
                        nc.scalar.activation(
                            out=ou, in_=o_tiles[qc], func=AF.Copy
                        )
                    z_ps = sps.tile([128, 512], F32, tag="sps", name=f"z{qc}")
                    nc.tensor.matmul(
                        out=z_ps, lhsT=waug, rhs=ou, start=True, stop=True
                    )
                    if qc % 2:
                        nc.scalar.activation(
                            out=z_all[0 : C + 1, bass.ts(qc, 512)],
                            in_=z_ps[0 : C + 1, :],
                            func=AF.Copy,
                        )
                    else:
                        nc.vector.tensor_copy(
                            out=z_all[0 : C + 1, bass.ts(qc, 512)],
                            in_=z_ps[0 : C + 1, :],
                        )
                heng = nc.scalar if hh else nc.sync
                jsl = bass.ds(hh * 8, 8)
                heng.dma_start_transpose(
                    out=zt_all[:, jsl, 0:80],
                    in_=z_all[:, bass.ds(hh * 1024, 1024)],
                )
                nc.vector.reciprocal(
                    out=r_all[:, jsl], in_=zt_all[:, jsl, C]
                )
                for j in range(8 * hh, 8 * hh + 8):
                    nc.vector.scalar_tensor_tensor(
                        out=y_all[:, j, :], in0=zt_all[:, j, 0:C],
                        scalar=r_all[:, j : j + 1], in1=xtb[:, j, :],
                        op0=ALU.mult, op1=ALU.add,
                    )
                heng.dma_start(out=y_view[:, jsl, :], in_=y_all[:, jsl, :])
    return nc


_NC = None


def _get_nc():
    global _NC
    if _NC is None:
        _NC = build_nc()
    return _NC


def _prep_maps(x, Wq, bq, Wk, bk, Wv, bv, Wo, bo, gamma, beta):
    bf = ml_dtypes.bfloat16
    w_qk = np.concatenate([Wq.T, Wk.T], axis=1).astype(bf)  # [cin, 2c]
    wv_t = np.ascontiguousarray(Wv.T).astype(bf)
    w_aug = np.zeros((C + 1, C + 1), np.float32)
    w_aug[:C, :C] = Wo.T
    w_aug[C, C] = 1.0
    w_aug = w_aug.astype(bf)
    gbias = np.zeros((2 * C, 4), np.float32)
    gbias[:C, 0] = gamma
    gbias[:C, 1] = beta
    gbias[:, 2] = np.concatenate([bq, bk])
    gbias[:C, 3] = bv
    brows = np.zeros((1, 3 * C), np.float32)
    brows[0, : 2 * C] = np.concatenate([bq, bk])
    brows[0, 2 * C :] = bv
    bo_bc = np.tile(bo[None, :], (128, 1)).astype(np.float32)

    shared = dict(
        w_qk=w_qk, wv_t=wv_t, w_aug=w_aug, gbias=gbias, brows=brows,
        bo_bc=bo_bc, zpad=np.zeros((C, N), bf),
    )
    in_maps = []
    for core in range(8):
        b, half = core // 2, core % 2
        xm = np.ascontiguousarray(x[b].reshape(C, N)).astype(np.float32)
        ones = np.ones((1, N), np.float32)
        xm1 = np.concatenate([xm, ones], axis=0)
        xqm = np.ascontiguousarray(xm1[:, half * NQ : (half + 1) * NQ])
        xtm = np.ascontiguousarray(xm.T[half * NQ : (half + 1) * NQ, :])
        in_maps.append(dict(shared, x=xm1, xq=xqm, xt=xtm))
    return in_maps


def run(inputs, trace=False):
    from concourse.bass_utils import run_bass_kernel_spmd

    inputs = {k: np.asarray(v) for k, v in inputs.items()}
    nc = _get_nc()
    in_maps = _prep_maps(**inputs)
    res = run_bass_kernel_spmd(
        nc, in_maps, core_ids=list(range(8)), trace=trace
    )
    out = np.empty((B, C, N), np.float32)
    for core in range(8):
        b, half = core // 2, core % 2
        out[b][:, half * NQ : (half + 1) * NQ] = res.results[core]["y"].T
    return out.reshape(B, C, H, W), res


def kernel(**inputs):
    out, _ = run(inputs, trace=False)
    return out

